# revision 82
# baseline (speedup 1.0000x reference)
"""Trainium2 Bass kernel for nn_Encoder (dense transformer block), 8 NeuronCores.

Two-phase plan built around hiding the ACT-engine exp pass (the irreducible
~133 us/core softmax cost) underneath the FFN matmuls:

  Phase A (head-parallel, tiny): core c computes q,k,v projections for its
    2 heads over all 4096 tokens in fp8 DoubleRow matmuls (~27 us).
  Host: pure byte-level reshuffles of the fp8 q/k/v into the layouts phase B
    wants (no arithmetic).
  Phase B (token-parallel): core c runs full attention (all 16 heads) for its
    512 query tokens + Wo proj + LN1 + FFN + LN2. Tokens are processed in 4
    sub-chunks of 128, software-pipelined so subchunk c's scores/exp/ctx
    (PE+ACT+DVE) overlap subchunk c-1's FFN matmuls (PE) - exp runs in the
    shadow of the FFN.

Scheduling notes (what the ~100us over the baseline came from):
  - The exp stream is sacred: every ACT op other than exp is either removed
    (LN1 needs no rstd: with trivial affine+b1=0 the FFN is positively
    homogeneous per token, so y2 = rstd1*(f'+z1') and LN2 cancels rstd1
    exactly) or emitted only after its inputs are compute-complete (LN2's
    rstd rides in finish_b, two jobs after finish_a computed the variance),
    because the strict-FIFO ACT queue otherwise head-of-line-blocks exps.
  - att+ln1 of subchunk c jump the job queue at the boundary (att's PSUM
    lives in a spool slot, not the FFN accumulator), so z1(c) is ready long
    before ffn(c) quarters drain.
  - FFN2(og) is ordered one og-slot behind FFN1(og+1) (h8 double-buffered)
    so the DVE relu latency hides under FFN1 matmuls.
  - w1 streams in og-column slices so weight arrival order matches FFN1's
    consumption order; subchunk input loads ride the Pool SWDGE queue so the
    bulk weight stream cannot delay them.
  - The softmax floor clamp is a single merged DVE op per step; the ctx
    normalize is a plain tensor_mul (the 8x ctx scale is folded into the 1/8
    ones row of vp on the host), split Pool/DVE by output partition half.

Precision: attention contributes only ~8.5% of the output (residual dominates)
so the whole attention path runs in fp8e4m3 (measured end-to-end rel err
~8.7e-3 vs the 2e-2 budget); the FFN runs in bf16 (fp8 FFN measured at
2.2-3.3e-2 - over budget). fp8 DoubleRow matmuls pair two 128-deep
contraction slices at 0.5 cycles/row.

Scaling ledger (to keep every fp8 tensor in e4m3's comfortable range):
  W*16 on host -> q',k',v' = 16x natural, scores s' = 256*s
  exp(s'/2048 - 6ln2) = 2^-6 * exp(s/8), clamped below at 2^-6 (the
  relu-softmax identity softmax(relu(s)) with exp(relu(x)) = max(exp(x),1))
  ctx8 = 8 * c[0:64]/c[64] = 128x natural (the 8x from the 1/8 ones row);
  att_psum = ctx8 @ (Wo.T*16) = 2048x natural -> y1 = att_psum/2048 + X.
"""

import os
import sys

for _p in ("/opt/trn_rl_repo",):
    if _p not in sys.path:
        sys.path.insert(0, _p)

_jp = os.environ.get("JAX_PLATFORMS")
if _jp is not None and "axon" not in _jp:
    os.environ["JAX_PLATFORMS"] = "axon," + _jp

import math

import ml_dtypes
import numpy as np

import concourse.bass as bass
import concourse.mybir as mybir
import concourse.tile as tile
from concourse import bacc
from concourse.bass_utils import run_bass_kernel_spmd

F32 = mybir.dt.float32
F8 = mybir.dt.float8e4
BF16 = mybir.dt.bfloat16
AF = mybir.ActivationFunctionType
OP = mybir.AluOpType
DRM = mybir.MatmulPerfMode.DoubleRow
E4 = ml_dtypes.float8_e4m3
BF = ml_dtypes.bfloat16

N_CORES = 8
B, S, D, H, DH, FF = 2, 2048, 1024, 16, 64, 4096
N = B * S            # 4096 tokens
P = 128
QC = N // N_CORES    # 512 tokens per core in phase B
NSUB = 4             # phase B token sub-chunks
QS = 128             # max subchunk size (PSUM tiles padded to this)
QSL = [128, 128, 128, 128]         # per-subchunk query counts (sum = QC)
OFF = [0, 128, 256, 384]           # per-subchunk token offsets
KI = S // P          # 16 key chunks per batch
EPS = 1e-5

WSC = 16.0                       # host weight scale for fp8
EXP_SCALE = 1.0 / 2048.0         # s' = 256*s ; s~ = s/8  -> 1/(8*256)
EXP_BIAS = float(-6.0 * math.log(2.0))   # probs scaled by 2^-6
PFLOOR = 2.0 ** -6
CTX_SCALE = 8.0                  # ctx8 = 8 * normalized ctx' (folded into the
#                                  1/8 ones row of vp on the host)
ATT_INV = 1.0 / 2048.0           # 1/(WSC*WSC*CTX_SCALE)

_CACHE = {}


# --------------------------------------------------------------------------
# Phase A: fp8 q/k/v projections for this core's 2 heads, all 4096 tokens.
#   xt8 [D, N]  X^T in fp8 (replicated)
#   wq8/wk8 [D, 128]  cols = [h0 dh0-31 | h1 dh0-31 | h0 dh32-63 | h1 dh32-63]
#   wv8 [D, 128]      cols = [h0 dh0-63 | h1 dh0-63]          (all x16)
# Outputs q8/k8 [128, N] rows in the same split-half order; v8 rows natural.
# --------------------------------------------------------------------------
def _build_phase_a():
    nc = bacc.Bacc("TRN2", target_bir_lowering=False, debug=False,
                   num_devices=N_CORES)
    xt8 = nc.dram_tensor("xt8", [D, N], F8, kind="ExternalInput")
    # host-pretiled [p, j, t, m] so the load is 1KB-contiguous per partition
    wq8 = nc.dram_tensor("wq8", [P, 4 * 2 * P], F8, kind="ExternalInput")
    wk8 = nc.dram_tensor("wk8", [P, 4 * 2 * P], F8, kind="ExternalInput")
    wv8 = nc.dram_tensor("wv8", [P, 4 * 2 * P], F8, kind="ExternalInput")
    q8 = nc.dram_tensor("q8", [P, N], F8, kind="ExternalOutput")
    k8 = nc.dram_tensor("k8", [P, N], F8, kind="ExternalOutput")
    v8 = nc.dram_tensor("v8", [P, N], F8, kind="ExternalOutput")
    CH = 512       # token columns per chunk (512B runs: full DMA rate)
    NO = N // CH   # 8 chunks

    with tile.TileContext(nc) as tc:
        with (
            tc.tile_pool(name="persist", bufs=1) as persist,
            tc.tile_pool(name="xp", bufs=3) as xp,
            tc.tile_pool(name="op", bufs=2) as op,
            tc.tile_pool(name="ps", bufs=2, space="PSUM") as ps,
        ):
            wq_sb = persist.tile([P, 4, 2, P], F8)
            wk_sb = persist.tile([P, 4, 2, P], F8)
            wv_sb = persist.tile([P, 4, 2, P], F8)
            for t_sb, t_dr in ((wq_sb, wq8), (wk_sb, wk8), (wv_sb, wv8)):
                nc.sync.dma_start(
                    t_sb[:],
                    t_dr.ap().rearrange("p (j t m) -> p j t m", j=4, t=2))

            state_o = []
            for o in range(NO):
                sl = slice(o * CH, (o + 1) * CH)
                xt_sb = xp.tile([P, 4, 2, CH], F8, name="xt")
                nc.sync.dma_start(
                    xt_sb[:],
                    xt8[:, sl].rearrange("(j t p) q -> p j t q", p=P, t=2))
                q_ps = ps.tile([P, CH], F32, name="q_ps")
                k_ps = ps.tile([P, CH], F32, name="k_ps")
                v_ps = ps.tile([P, CH], F32, name="v_ps")
                for w_sb, o_ps in ((wq_sb, q_ps), (wk_sb, k_ps),
                                   (wv_sb, v_ps)):
                    for j in range(4):
                        nc.tensor.matmul(o_ps[:], w_sb[:, j], xt_sb[:, j],
                                         start=(j == 0), stop=(j == 3),
                                         perf_mode=DRM)
                if o % 2 == 0:
                    outs = [op.tile([P, 2, CH], F8, name=nm)
                            for nm in ("qo", "ko", "vo")]
                    state_o.clear()
                    state_o.extend(outs)
                qo, ko, vo = state_o
                nc.vector.tensor_copy(qo[:, o % 2, :], q_ps[:])
                nc.scalar.activation(ko[:, o % 2, :], k_ps[:], AF.Copy)
                nc.vector.tensor_copy(vo[:, o % 2, :], v_ps[:])
                if o % 2 == 1:
                    # SWDGE queue: an out-DMA waiting on the copies must not
                    # head-of-line-block the xt input stream on the sync queue
                    sl2 = slice((o - 1) * CH, (o + 1) * CH)
                    nc.gpsimd.dma_start(q8[:, sl2], qo[:])
                    nc.gpsimd.dma_start(k8[:, sl2], ko[:])
                    nc.gpsimd.dma_start(v8[:, sl2], vo[:])
    nc.compile()
    return nc


# --------------------------------------------------------------------------
# Phase B: token-parallel attention + Wo + LN1 + FFN + LN2 for 512 tokens.
# Layouts (host-assembled, all partition-major 2D drams):
#   qt8 [128, 4*2*QC]   qt[32*bb+r, g, t, q] = q'[head 4g+bb, dh 32t+r, tok q]
#   kt8 [128, 4*2*S]    same over the batch's 2048 keys
#   vp8 [128, KI*H*65]  vp[p, kc, h, m<64] = v'[h, m, key 128kc+p]; [...,64]=1
#   wot8 [128, 4*2*D]   wot[p,j,t,m] = 16*Wo.T[64*(4j+2t+p//64)+p%64, m]
#   w1t [D, FF] bf16 (W1.T), w2t [FF, D] bf16 (W2.T), xts [D, QC] bf16 (X.T)
#   lnp [128, 40] f32: cols g1(8) be1(8) g2(8) be2(8) b2(8);  b1t [128,32] f32
# Output out_t [D, QC] f32.
# --------------------------------------------------------------------------
def _build_phase_b(trivial_affine):
    nc = bacc.Bacc("TRN2", target_bir_lowering=False, debug=False,
                   num_devices=N_CORES)
    qt8 = nc.dram_tensor("qt8", [P, 4 * 2 * QC], F8, kind="ExternalInput")
    kt8 = nc.dram_tensor("kt8", [P, 4 * 2 * S], F8, kind="ExternalInput")
    vp8 = nc.dram_tensor("vp8", [P, KI * H * 65], F8, kind="ExternalInput")
    wot8 = nc.dram_tensor("wot8", [P, 4 * 2 * D], F8, kind="ExternalInput")
    w1t = nc.dram_tensor("w1t", [D, FF], BF16, kind="ExternalInput")
    w2t = nc.dram_tensor("w2t", [FF, D], BF16, kind="ExternalInput")
    xts = nc.dram_tensor("xts", [D, QC], BF16, kind="ExternalInput")
    lnp = nc.dram_tensor("lnp", [P, 40], F32, kind="ExternalInput")
    b1t = nc.dram_tensor("b1t", [P, 32], F32, kind="ExternalInput")
    out_t = nc.dram_tensor("out_t", [D, QC], F32, kind="ExternalOutput")

    with tile.TileContext(nc) as tc:
        with (
            tc.tile_pool(name="persist", bufs=1) as persist,
            tc.tile_pool(name="xpool", bufs=2) as xpool,
            tc.tile_pool(name="zpool", bufs=2) as zpool,
            tc.tile_pool(name="hpool8", bufs=2) as hpool8,
            tc.tile_pool(name="slabp", bufs=3) as slabp,
            tc.tile_pool(name="ypool", bufs=2) as ypool,
            tc.tile_pool(name="sqp", bufs=1) as sqp,
            tc.tile_pool(name="craw", bufs=1) as craw,
            tc.tile_pool(name="ctxp", bufs=1) as ctxp,
            tc.tile_pool(name="outp", bufs=1) as outp,
            tc.tile_pool(name="bcp", bufs=2) as bcp,
            tc.tile_pool(name="bcb", bufs=1) as bcb,
            tc.tile_pool(name="smallp", bufs=1) as smallp,
            tc.tile_pool(name="spool", bufs=2, space="PSUM") as spool,
            tc.tile_pool(name="cpool", bufs=1, space="PSUM") as cpool,
            tc.tile_pool(name="fpool", bufs=1, space="PSUM") as fpool,
            tc.tile_pool(name="hpool", bufs=1, space="PSUM") as hpool,
        ):
            kt_sb = persist.tile([P, 4, 2, S], F8)

            vp_sb = persist.tile([P, KI, H, 65], F8)
            wot_sb = persist.tile([P, 4, 2, D], F8)
            w1_sb = persist.tile([P, 8, FF], BF16)
            w2_sb = persist.tile([P, 32, D], BF16)
            lnp_sb = persist.tile([P, 40], F32)
            b1_sb = persist.tile([P, 32], F32)
            ones_c = persist.tile([P, 1], BF16)
            ebias = persist.tile([P, 1], F32)
            epst = persist.tile([P, 1], F32)

            def load_persist():
                ktr = kt8.ap().rearrange("p (g t k) -> p g t k", g=4, t=2)
                vpr = vp8.ap().rearrange("p (kc h m) -> p kc h m",
                                         kc=KI, h=H)
                nc.sync.dma_start(kt_sb[:, 0], ktr[:, 0])
                # ctx(0,0) at ~10us contracts over ALL 16 vp key-chunks, so
                # the whole vp tensor must land before the later kt groups
                nc.sync.dma_start(vp_sb[:, 0:8], vpr[:, 0:8])
                nc.sync.dma_start(vp_sb[:, 8:KI], vpr[:, 8:KI])
                for g in range(1, 4):
                    nc.sync.dma_start(kt_sb[:, g], ktr[:, g])
                nc.sync.dma_start(lnp_sb[:], lnp.ap())
                nc.sync.dma_start(b1_sb[:], b1t.ap())
            nc.vector.memset(ones_c[:], 1.0)
            nc.vector.memset(ebias[:], EXP_BIAS)
            nc.vector.memset(epst[:], EPS)

            def w1_og(og):
                # w1 streams in og-column slices: FFN1(og) needs exactly one
                # slice (all kc rows of its 512 hidden columns), so arrival
                # order matches consumption order.
                nc.sync.dma_start(
                    w1_sb[:, :, og * 512:(og + 1) * 512],
                    w1t[:, og * 512:(og + 1) * 512].rearrange(
                        "(kc p) m -> p kc m", p=P))

            def w2_fk(j):
                nc.sync.dma_start(
                    w2_sb[:, 4 * j:4 * j + 4],
                    w2t[4 * j * P:(4 * j + 4) * P, :].rearrange(
                        "(f p) m -> p f m", p=P))

            def stream_weights():
                for og in range(4):
                    w1_og(og)
                w2_fk(0)
                nc.sync.dma_start(
                    wot_sb[:], wot8.ap().rearrange("p (j t m) -> p j t m",
                                                   j=4, t=2))
                for og in range(4, 8):
                    w1_og(og)
                    w2_fk(og - 3)
                for j in range(5, 8):
                    w2_fk(j)

            g1c = lambda kc: lnp_sb[:, kc:kc + 1]
            be1c = lambda kc: lnp_sb[:, 8 + kc:9 + kc]
            g2c = lambda kc: lnp_sb[:, 16 + kc:17 + kc]
            be2c = lambda kc: lnp_sb[:, 24 + kc:25 + kc]
            b2c = lambda kc: lnp_sb[:, 32 + kc:33 + kc]

            state = {}

            def scores_half(c, h, half):
                """scores matmuls + exp for one 8-key-chunk half of head h.
                PSUM zero-region rule: one start=True per 2KB bank region.
                After half 1, the relu-softmax floor clamp runs over the whole
                slab, alternating DVE/Pool per step to balance engine load."""
                qs = QSL[c]
                g, bb = h // 4, h % 4
                pp = slice(32 * bb, 32 * bb + 32)
                if half == 0:
                    state[c]["slab"][h] = slabp.tile([P, 16, qs], F8,
                                                     name="slab")
                slab = state[c]["slab"][h]
                s_ps = spool.tile([P, 8, QS], F32, name="s_ps")
                for kc8 in range(8):
                    kc = half * 8 + kc8
                    nc.tensor.matmul(
                        s_ps[:, kc8, 0:qs],
                        kt_sb[pp, g, :, kc * P:(kc + 1) * P],
                        state[c]["qt"][pp, g, :, :],
                        start=(kc8 % 4 == 0), stop=True,
                        perf_mode=DRM, skip_group_check=True,
                        tile_position=(32 * bb, 0))
                hsl = slice(8 * half, 8 * half + 8)
                nc.scalar.activation(slab[:, hsl, :], s_ps[:, :, 0:qs],
                                     AF.Exp, scale=EXP_SCALE, bias=ebias[:])
                if half == 1:
                    nc.vector.tensor_scalar_max(slab[:], slab[:], PFLOOR)

            def ctx_part(c, h):
                """ctx accumulation over keys + stage raw result to SBUF."""
                qs = QSL[c]
                slab = state[c]["slab"].pop(h)
                c_ps = cpool.tile([65, 512], F32, name="c_ps")
                for pj in range(8):
                    nc.tensor.matmul(
                        c_ps[:, 0:qs],
                        vp_sb[:, 2 * pj:2 * pj + 2, h, :],
                        slab[:, 2 * pj:2 * pj + 2, :],
                        start=(pj == 0), stop=(pj == 7),
                        perf_mode=DRM, skip_group_check=True)
                nc.vector.tensor_copy(state[c]["craw"][:, h, :],
                                      c_ps[:, 0:qs])

            def norm_group(c, grp):
                """Normalize heads 8*grp..8*grp+7 of subchunk c into ctx fp8.
                The ones row of vp is 1/8 on the host, so 1/craw[64] already
                carries the 8x ctx scale; normalization is a plain multiply.
                sig=0 keeps partitions aligned (Pool-safe); sig=1 shifts the
                output partition base, which only DVE handles."""
                qs = QSL[c]
                cr = state[c]["craw"]
                hs = slice(8 * grp, 8 * grp + 8)
                inv = bcb.tile([1, 8, qs], BF16, name="inv")
                with nc.allow_low_precision(reason="fp8 attention path"):
                    nc.vector.reciprocal(inv[:], cr[64:65, hs, :])
                invb = bcb.tile([64, 8, qs], BF16, name="invb")
                nc.gpsimd.partition_broadcast(invb[:], inv[:])
                ctx = state[c]["ctx"]
                for sig, eng in ((0, nc.gpsimd), (1, nc.vector)):
                    eng.tensor_mul(
                        ctx[64 * sig:64 * sig + 64, 4 * grp:4 * grp + 4, :],
                        cr[0:64, 8 * grp + sig:8 * grp + 8:2, :],
                        invb[:, sig::2, :])

            def rstd_via_lnexp(dst_rstd, var_ap):
                # rstd = exp(-0.5 * ln(var + eps)); stays in the exp/ln
                # ACT table set so no table reloads are ever needed.
                nc.scalar.activation(dst_rstd, var_ap, AF.Ln, bias=epst[0:1, :])
                nc.scalar.activation(dst_rstd, dst_rstd, AF.Exp, scale=-0.5)

            def layer_norm(y_sb, out_sb, gc, bec, qs):
                """t-layout LN over the 1024 partition-features.
                y_sb [P, 8, qs] bf16 -> out_sb (bf16 or f32)."""
                st = hpool.tile([P, 4, QS], F32, name="h_ps")
                # two accumulation chains on DIFFERENT partitions (0 and 32):
                # PSUM zero regions are per-partition, so both chains can use
                # clean start=True semantics (no lazy-zero assumptions).
                for kc in range(8):
                    nc.tensor.matmul(st[0:1, 0, 0:qs], ones_c[:],
                                     y_sb[:, kc, :],
                                     start=(kc == 0), stop=(kc == 7),
                                     skip_group_check=True)
                for hf in (0, 1):
                    sq = sqp.tile([P, 4, qs], BF16, name="sq")
                    nc.vector.tensor_mul(sq[:], y_sb[:, 4 * hf:4 * hf + 4, :],
                                         y_sb[:, 4 * hf:4 * hf + 4, :])
                    for k4 in range(4):
                        nc.tensor.matmul(st[32:33, 0, 0:qs], ones_c[:],
                                         sq[:, k4, :], start=(hf + k4 == 0),
                                         stop=(hf == 1 and k4 == 3),
                                         skip_group_check=True)
                stats = smallp.tile([1, 2, qs], F32, name="stats")
                stats_bf = smallp.tile([1, 2, qs], BF16, name="stats_bf")
                nc.vector.tensor_scalar_mul(stats[0:1, 0, :],
                                            st[0:1, 0, 0:qs], 1.0 / D)
                nc.vector.tensor_scalar_mul(stats[0:1, 1, :],
                                            st[32:33, 0, 0:qs], 1.0 / D)
                mean = stats[0:1, 0, :]
                nc.vector.tensor_mul(stats_bf[0:1, 0, :], mean, mean)
                nc.vector.tensor_sub(stats[0:1, 1, :], stats[0:1, 1, :],
                                     stats_bf[0:1, 0, :])       # var
                rstd_via_lnexp(stats[0:1, 1, :], stats[0:1, 1, :])
                nc.vector.tensor_mul(stats_bf[0:1, 1, :], mean,
                                     stats[0:1, 1, :])          # ms (bf16)
                nc.vector.tensor_copy(stats_bf[0:1, 0, :], stats[0:1, 1, :])
                for hf in (0, 1):
                    veng = nc.vector
                    rstd_b = bcb.tile([P, 4, qs], BF16, name="rstd_b")
                    ms_b = bcb.tile([P, 4, qs], BF16, name="ms_b")
                    for k4 in range(4):
                        nc.gpsimd.partition_broadcast(rstd_b[:, k4, :],
                                                      stats_bf[0:1, 0, :])
                        nc.gpsimd.partition_broadcast(ms_b[:, k4, :],
                                                      stats_bf[0:1, 1, :])
                    ks = slice(4 * hf, 4 * hf + 4)
                    tmp = sqp.tile([P, 4, qs], BF16, name="sq")
                    veng.tensor_mul(tmp[:], y_sb[:, ks, :], rstd_b[:])
                    if trivial_affine:
                        veng.tensor_sub(out_sb[:, ks, :], tmp[:], ms_b[:])
                    else:
                        nc.vector.tensor_sub(tmp[:], tmp[:], ms_b[:])
                        for k4 in range(4):
                            kc = 4 * hf + k4
                            nc.vector.tensor_scalar(
                                out=out_sb[:, kc, :], in0=tmp[:, k4, :],
                                scalar1=gc(kc), scalar2=bec(kc),
                                op0=OP.mult, op1=OP.add)

            def att_mm_y1(c):
                """Wo projection (fp8 DR) + residual -> y1[c] (bf16)."""
                qs = QSL[c]
                st = state[c]
                a_ps = spool.tile([P, 8, QS], F32, name="s_ps")
                for mc in range(8):
                    for j in range(4):
                        nc.tensor.matmul(
                            a_ps[:, mc, 0:qs],
                            wot_sb[:, j, :, mc * P:(mc + 1) * P],
                            st["ctx"][:, 2 * j:2 * j + 2, :],
                            start=(j == 0 and mc % 4 == 0), stop=(j == 3),
                            perf_mode=DRM, skip_group_check=True)
                y1 = ypool.tile([P, 8, qs], BF16, name="y1")
                nc.vector.scalar_tensor_tensor(
                    out=y1[:], in0=a_ps[:, :, 0:qs], scalar=ATT_INV,
                    in1=st["xts"][:], op0=OP.mult, op1=OP.add)
                st["y1"] = y1

            def ln1(c):
                """LN1 without rstd: with trivial affine and b1=0, the FFN is
                positively homogeneous per token (relu(a*x)=a*relu(x), a>0),
                so y2 = f + z1 = rstd1*(f' + z1') and LN2 cancels the rstd1
                exactly (eps shift ~1e-6 relative). Only the mean is removed.
                Falls back to the full LN when affine params are nontrivial."""
                qs = QSL[c]
                st = state[c]
                z1 = zpool.tile([P, 8, qs], BF16, name="z1")
                if not trivial_affine:
                    layer_norm(st["y1"], z1, g1c, be1c, qs)
                    st["z1"] = z1
                    return
                y = st["y1"]
                stq = hpool.tile([P, 4, QS], F32, name="h_ps")
                for kc in range(8):
                    nc.tensor.matmul(stq[0:1, 0, 0:qs], ones_c[:], y[:, kc, :],
                                     start=(kc == 0), stop=(kc == 7),
                                     skip_group_check=True)
                mu_bf = smallp.tile([1, 1, qs], BF16, name="mu_bf")
                nc.vector.tensor_scalar_mul(mu_bf[0:1, 0, :],
                                            stq[0:1, 0, 0:qs], 1.0 / D)
                mu_b = bcb.tile([P, 4, qs], BF16, name="mu_b")
                for k4 in range(4):
                    nc.gpsimd.partition_broadcast(mu_b[:, k4, :],
                                                  mu_bf[0:1, 0, :])
                for hf in (0, 1):
                    nc.vector.tensor_sub(z1[:, 4 * hf:4 * hf + 4, :],
                                         y[:, 4 * hf:4 * hf + 4, :], mu_b[:])
                st["z1"] = z1

            def ffn_quarter(c, qi):
                """FFN work in 16-matmul quarters: og = qi//4,
                quarter 0/1 = FFN1 halves (+relu), 2/3 = FFN2 halves.
                The caller orders FFN2(og) one og-slot behind FFN1(og+1) so
                the DVE relu latency hides under FFN1 matmuls; h8 is
                double-buffered per og to allow it."""
                qs = QSL[c]
                st = state[c]
                og, qq = qi // 4, qi % 4
                if qq in (0, 1):
                    if qq == 0:
                        st["h_ps"] = hpool.tile([P, 4, QS], F32, name="h_ps")
                    h_ps = st["h_ps"]
                    for mc4 in (2 * qq, 2 * qq + 1):
                        for kc in range(8):
                            nc.tensor.matmul(
                                h_ps[:, mc4, 0:qs],
                                w1_sb[:, kc, og * 512 + mc4 * P:
                                      og * 512 + (mc4 + 1) * P],
                                st["z1"][:, kc, :],
                                start=(mc4 == 0 and kc == 0),
                                stop=(kc == 7), skip_group_check=True)
                    if qq == 1:
                        h8 = hpool8.tile([P, 4, qs], BF16, name="h8")
                        if trivial_affine:
                            nc.vector.tensor_scalar_max(h8[:],
                                                        h_ps[:, :, 0:qs], 0.0)
                        else:
                            for mc4 in range(4):
                                nc.scalar.activation(
                                    h8[:, mc4, :], h_ps[:, mc4, 0:qs],
                                    AF.Relu,
                                    bias=b1_sb[:, og * 4 + mc4:
                                               og * 4 + mc4 + 1])
                        st["h8"][og] = h8
                else:
                    if og == 0 and qq == 2:
                        st["f_ps"] = fpool.tile([P, 8, QS], F32, name="f_ps",
                                                tag="f_ps")
                    f_ps = st["f_ps"]
                    for i in (2 * (qq - 2), 2 * (qq - 2) + 1):
                        fk = og * 4 + i
                        for mc in range(8):
                            nc.tensor.matmul(
                                f_ps[:, mc, 0:qs],
                                w2_sb[:, fk, mc * P:(mc + 1) * P],
                                st["h8"][og][:, i, :],
                                start=(fk == 0 and mc % 4 == 0),
                                stop=(fk == 31), skip_group_check=True)
                    if qq == 3:
                        del st["h8"][og]

            def finish_a(c):
                """y2 = f + (b2) + z1 and LN2 stats through var - no ACT op.
                The ACT rstd goes in finish_b, enqueued a job later, so it
                never head-of-line-blocks the exp stream while var is still
                in flight."""
                qs = QSL[c]
                st = state[c]
                y2 = ypool.tile([P, 8, qs], BF16, name="y1")
                if trivial_affine:
                    nc.vector.tensor_add(y2[:], st["f_ps"][:, :, 0:qs],
                                         st["z1"][:])
                else:
                    for mc in range(8):
                        nc.vector.scalar_tensor_tensor(
                            out=y2[:, mc, :], in0=st["f_ps"][:, mc, 0:qs],
                            scalar=b2c(mc), in1=st["z1"][:, mc, :],
                            op0=OP.add, op1=OP.add)
                st["y2"] = y2
                stq = hpool.tile([P, 4, QS], F32, name="h_ps")
                for kc in range(8):
                    nc.tensor.matmul(stq[0:1, 0, 0:qs], ones_c[:],
                                     y2[:, kc, :],
                                     start=(kc == 0), stop=(kc == 7),
                                     skip_group_check=True)
                for hf in (0, 1):
                    sq = sqp.tile([P, 4, qs], BF16, name="sq")
                    nc.vector.tensor_mul(sq[:], y2[:, 4 * hf:4 * hf + 4, :],
                                         y2[:, 4 * hf:4 * hf + 4, :])
                    for k4 in range(4):
                        nc.tensor.matmul(stq[32:33, 0, 0:qs], ones_c[:],
                                         sq[:, k4, :], start=(hf + k4 == 0),
                                         stop=(hf == 1 and k4 == 3),
                                         skip_group_check=True)
                stats = smallp.tile([1, 2, qs], F32, name="stats")
                stats_bf = smallp.tile([1, 2, qs], BF16, name="stats_bf")
                nc.vector.tensor_scalar_mul(stats[0:1, 0, :],
                                            stq[0:1, 0, 0:qs], 1.0 / D)
                nc.vector.tensor_scalar_mul(stats[0:1, 1, :],
                                            stq[32:33, 0, 0:qs], 1.0 / D)
                mean = stats[0:1, 0, :]
                nc.vector.tensor_mul(stats_bf[0:1, 0, :], mean, mean)
                nc.vector.tensor_sub(stats[0:1, 1, :], stats[0:1, 1, :],
                                     stats_bf[0:1, 0, :])       # var
                st["ln2"] = (stats, stats_bf)

            def finish_b(c):
                """rstd (ACT), normalize, write out subchunk c."""
                qs = QSL[c]
                st = state[c]
                y2 = st["y2"]
                stats, stats_bf = st["ln2"]
                mean = stats[0:1, 0, :]
                rstd_via_lnexp(stats[0:1, 1, :], stats[0:1, 1, :])
                nc.vector.tensor_mul(stats_bf[0:1, 1, :], mean,
                                     stats[0:1, 1, :])          # ms (bf16)
                nc.vector.tensor_copy(stats_bf[0:1, 0, :], stats[0:1, 1, :])
                for hf in (0, 1):
                    rstd_b = bcb.tile([P, 4, qs], BF16, name="rstd_b")
                    ms_b = bcb.tile([P, 4, qs], BF16, name="ms_b")
                    for k4 in range(4):
                        nc.gpsimd.partition_broadcast(rstd_b[:, k4, :],
                                                      stats_bf[0:1, 0, :])
                        nc.gpsimd.partition_broadcast(ms_b[:, k4, :],
                                                      stats_bf[0:1, 1, :])
                    ks = slice(4 * hf, 4 * hf + 4)
                    z2 = outp.tile([P, 4, qs], F32, name="z2")
                    tmp = sqp.tile([P, 4, qs], BF16, name="sq")
                    nc.vector.tensor_mul(tmp[:], y2[:, ks, :], rstd_b[:])
                    if trivial_affine:
                        nc.vector.tensor_sub(z2[:], tmp[:], ms_b[:])
                    else:
                        nc.vector.tensor_sub(tmp[:], tmp[:], ms_b[:])
                        for k4 in range(4):
                            kc = 4 * hf + k4
                            nc.vector.tensor_scalar(
                                out=z2[:, k4, :], in0=tmp[:, k4, :],
                                scalar1=g2c(kc), scalar2=be2c(kc),
                                op0=OP.mult, op1=OP.add)
                    nc.sync.dma_start(
                        out_t[4 * hf * P:4 * hf * P + 4 * P,
                              OFF[c]:OFF[c] + qs].rearrange(
                            "(kc p) q -> p kc q", p=P), z2[:])
                del state[c]

            # ---- global software-pipelined stream over 64 head-steps ----
            # PE emission per step: ctx(step-3) | scores half 0 | ~1.2us of
            # queued jobs | scores half 1 | rest of the job quantum. The
            # scores halves single-buffer through one 2-bank PSUM slot; the
            # job quantum between them covers the exp WAR latency. Boundary
            # work (norms, Wo+LN1, FFN quarters, LN2+out) drains from the job
            # queue paced by estimated PE-ns so the exp stream never starves.
            jobs = []

            def drain(budget_ns, keep_min=0):
                spent, n = 0, 0
                while len(jobs) > keep_min and spent < budget_ns and n < 8:
                    cost, fn = jobs.pop(0)
                    fn()
                    spent += cost
                    n += 1

            def load_inputs(c, eng=None):
                # later subchunks load via the Pool SWDGE queue so they are
                # not stuck behind the bulk w1/w2 stream on the sync queue
                qs = QSL[c]
                eng = eng or nc.sync
                xts_sb = xpool.tile([P, 8, qs], BF16, name="xts")
                eng.dma_start(
                    xts_sb[:],
                    xts[:, OFF[c]:OFF[c] + qs].rearrange(
                        "(kc p) q -> p kc q", p=P))
                qt_c = xpool.tile([P, 4, 2, qs], F8, name="qt")
                eng.dma_start(
                    qt_c[:],
                    qt8.ap().rearrange("p (g t q) -> p g t q", g=4, t=2)[
                        :, :, :, OFF[c]:OFF[c] + qs])
                state[c] = {
                    "qt": qt_c,
                    "ctx": ctxp.tile([P, 8, qs], F8, name="ctx"),
                    "craw": craw.tile([65, H, qs], BF16, name="craw"),
                    "xts": xts_sb,
                    "slab": {},
                    "h8": {},
                }

            def enqueue_ffn(cp):
                # FFN2(og) trails FFN1(og+1) so relu latency hides
                order = []
                for og in range(8):
                    order += [4 * og, 4 * og + 1]
                    if og >= 1:
                        order += [4 * (og - 1) + 2, 4 * (og - 1) + 3]
                order += [30, 31]
                jc = int(860 * QSL[cp] / 128)
                for n_q, qi in enumerate(order):
                    jobs.append(
                        (jc, lambda cp=cp, qi=qi: ffn_quarter(cp, qi)))
                    if n_q == 1 and cp >= 1:
                        # rstd of LN2(cp-1): two quarters after finish_a so
                        # var is computed before the Ln lands on ACT
                        jobs.append((200, lambda cp=cp: finish_b(cp - 1)))
                jobs.append((jc, lambda cp=cp: finish_a(cp)))

            load_inputs(0)
            load_persist()
            load_inputs(1)
            stream_weights()

            NSTEP = NSUB * H
            for s in range(NSTEP + 3):
                if s >= 3:
                    cp, hp = divmod(s - 3, H)
                    ctx_part(cp, hp)
                    if hp == 7:
                        jobs.insert(0, (0, lambda cp=cp: norm_group(cp, 0)))
                    elif hp == 15:
                        norm_group(cp, 1)
                        # att+ln1 jump the queue so z1(cp) is ready before
                        # ffn(cp) quarters drain; att's a_ps lives in a spool
                        # slot so it does not wait on finish_a(cp-1)'s f_ps
                        jobs.insert(0, (900, lambda cp=cp: att_mm_y1(cp)))
                        jobs.insert(min(3, len(jobs)),
                                    (900, lambda cp=cp: ln1(cp)))
                        enqueue_ffn(cp)
                if s < NSTEP:
                    c, h = divmod(s, H)
                    drain(1600)
                    scores_half(c, h, 0)
                    scores_half(c, h, 1)
                    if h == 15 and c + 2 < NSUB:
                        load_inputs(c + 2, eng=nc.gpsimd)
                else:
                    drain(2400)
            while jobs:
                drain(10**9)
            finish_b(NSUB - 1)
    _pin_act_tables(nc)
    nc.compile()
    return nc


def _pin_act_tables(nc):
    """Restrict the act-table-load pass to the one set that covers every
    ACT function this kernel uses (exp, ln, relu, copy), so exactly one
    table load is emitted instead of per-transition reloads."""
    import bass_rust as _bass_rust
    from concourse.hw_specs import get_activation_tables
    tabs = get_activation_tables(nc.m.arch)
    # keep every entry (act_func_set_id is the index into act_info.json's
    # list) but empty all other sets so the pass can only pick this one.
    only = [(k, (v if k == "natural_log_exp_and_others" else set()))
            for k, v in tabs.items()]
    if any(v for _, v in only):
        nc.insert_act_table_loads = (
            lambda: _bass_rust.insert_act_table_loads(nc, only))


def _get(name, builder, *args):
    if name not in _CACHE:
        _CACHE[name] = builder(*args)
    return _CACHE[name]


def _qcols(c):
    h0, h1 = 2 * c, 2 * c + 1
    r = np.arange(32)
    return np.concatenate([h0 * 64 + r, h1 * 64 + r,
                           h0 * 64 + 32 + r, h1 * 64 + 32 + r])


def _vcols(c):
    h0, h1 = 2 * c, 2 * c + 1
    r = np.arange(64)
    return np.concatenate([h0 * 64 + r, h1 * 64 + r])


def kernel(X, Wq, Wk, Wo, ln1_g, ln1_b, ln2_g, ln2_b, W1, b1, W2, b2):
    f32 = lambda a: np.asarray(a, np.float32)
    X, Wq, Wk, Wo, W1, W2 = map(f32, (X, Wq, Wk, Wo, W1, W2))
    ln1_g, ln1_b, ln2_g, ln2_b, b1, b2 = map(
        f32, (ln1_g, ln1_b, ln2_g, ln2_b, b1, b2))
    Xf = X.reshape(N, D)
    Xt8 = np.ascontiguousarray(Xf.T).astype(E4)          # [D, N]
    WqT, WkT, WoT = Wq.T, Wk.T, Wo.T

    # ---------------- phase A ----------------
    nc_a = _get("a", _build_phase_a)

    def _pretile(arr):   # [D, P] -> [p, (j t m)] with row (j*2+t)*128+p
        return np.ascontiguousarray(
            arr.reshape(4, 2, P, P).transpose(2, 0, 1, 3).reshape(P, 8 * P))

    in_a = []
    for c in range(N_CORES):
        qc, vc = _qcols(c), _vcols(c)
        in_a.append({
            "xt8": Xt8,
            "wq8": _pretile((WqT[:, qc] * WSC).astype(E4)),
            "wk8": _pretile((WkT[:, qc] * WSC).astype(E4)),
            "wv8": _pretile((WoT[:, vc] * WSC).astype(E4)),
        })
    res_a = run_bass_kernel_spmd(nc_a, in_a, core_ids=list(range(N_CORES)))

    # assemble full q/k/v byte arrays [H, DH, N] (uint8 views of e4m3)
    u8 = np.uint8
    qfull = np.empty((H, DH, N), u8)
    kfull = np.empty((H, DH, N), u8)
    vfull = np.empty((H, DH, N), u8)
    for c in range(N_CORES):
        r = res_a.results[c]
        qa = np.asarray(r["q8"]).view(u8).reshape(2, 2, 32, N)  # [i, j, r, :]
        ka = np.asarray(r["k8"]).view(u8).reshape(2, 2, 32, N)
        va = np.asarray(r["v8"]).view(u8).reshape(2, DH, N)     # [j, dh, :]
        for j in range(2):
            qfull[2 * c + j] = qa[:, j].reshape(DH, N)
            kfull[2 * c + j] = ka[:, j].reshape(DH, N)
            vfull[2 * c + j] = va[j]

    # ---------------- phase B host layouts ----------------
    trivial = (not b1.any()) and (not b2.any()) and \
        np.all(ln1_g == 1) and (not ln1_b.any()) and \
        np.all(ln2_g == 1) and (not ln2_b.any())
    nc_b = _get("b", _build_phase_b, trivial)

    w1t = np.ascontiguousarray(W1.astype(BF).T)          # [D, FF] bf16
    w2t = np.ascontiguousarray(W2.astype(BF).T)          # [FF, D] bf16
    one8 = np.float32(1.0 / CTX_SCALE).astype(E4).view(u8)  # ones row = 1/8
    wot_arr = (WoT * WSC).astype(E4).view(u8)            # [D, D]
    # wot8[p, j, t, m] = WoT*16[64*(4j+2t+p//64)+p%64, m]
    wot8 = wot_arr.reshape(4, 2, 2, 64, D).transpose(2, 3, 0, 1, 4) \
        .reshape(P, 4 * 2 * D).copy()
    lnp = np.zeros((P, 40), np.float32)
    for i, vvec in enumerate((ln1_g, ln1_b, ln2_g, ln2_b, b2)):
        lnp[:, 8 * i:8 * i + 8] = vvec.reshape(8, P).T
    b1t = np.ascontiguousarray(b1.reshape(32, P).T)

    in_b = []
    for c in range(N_CORES):
        toks = slice(c * QC, (c + 1) * QC)
        bb = (c * QC) // S
        keys = slice(bb * S, (bb + 1) * S)
        # qt8 [128, 4, 2, QC]: [bb*32+r, g, t, q] = qfull[4g+bb, 32t+r, tok]
        qx = qfull[:, :, toks].reshape(4, 4, 2, 32, QC)   # [g, bb, t, r, q]
        qt8 = qx.transpose(1, 3, 0, 2, 4).reshape(P, 4 * 2 * QC).copy()
        kx = kfull[:, :, keys].reshape(4, 4, 2, 32, S)
        kt8 = kx.transpose(1, 3, 0, 2, 4).reshape(P, 4 * 2 * S).copy()
        # vp8 [128, kc, h, 65]
        vv = vfull[:, :, keys].reshape(H, DH, KI, P).transpose(3, 2, 0, 1)
        vp8 = np.concatenate(
            [vv, np.full((P, KI, H, 1), one8, u8)], axis=3) \
            .reshape(P, KI * H * 65).copy()
        in_b.append({
            "qt8": qt8.view(E4), "kt8": kt8.view(E4), "vp8": vp8.view(E4),
            "wot8": wot8.view(E4),
            "w1t": w1t, "w2t": w2t,
            "xts": np.ascontiguousarray(Xf.T[:, toks]).astype(BF),
            "lnp": lnp, "b1t": b1t,
        })
    res_b = run_bass_kernel_spmd(nc_b, in_b, core_ids=list(range(N_CORES)))
    out_t = np.concatenate(
        [np.asarray(res_b.results[c]["out_t"]) for c in range(N_CORES)],
        axis=1)                                          # [D, N]
    return np.ascontiguousarray(out_t.T).reshape(B, S, D).astype(np.float32)



# revision 84
# speedup vs baseline: 1.0028x; 1.0028x over previous
"""Trainium2 Bass kernel for nn_Encoder (dense transformer block), 8 NeuronCores.

Two-phase plan built around hiding the ACT-engine exp pass (the irreducible
~133 us/core softmax cost) underneath the FFN matmuls:

  Phase A (head-parallel, tiny): core c computes q,k,v projections for its
    2 heads over all 4096 tokens in fp8 DoubleRow matmuls (~27 us).
  Host: pure byte-level reshuffles of the fp8 q/k/v into the layouts phase B
    wants (no arithmetic).
  Phase B (token-parallel): core c runs full attention (all 16 heads) for its
    512 query tokens + Wo proj + LN1 + FFN + LN2. Tokens are processed in 4
    sub-chunks of 128, software-pipelined so subchunk c's scores/exp/ctx
    (PE+ACT+DVE) overlap subchunk c-1's FFN matmuls (PE) - exp runs in the
    shadow of the FFN.

Scheduling notes (what the ~100us over the baseline came from):
  - The exp stream is sacred: every ACT op other than exp is either removed
    (LN1 needs no rstd: with trivial affine+b1=0 the FFN is positively
    homogeneous per token, so y2 = rstd1*(f'+z1') and LN2 cancels rstd1
    exactly) or emitted only after its inputs are compute-complete (LN2's
    rstd rides in finish_b, two jobs after finish_a computed the variance),
    because the strict-FIFO ACT queue otherwise head-of-line-blocks exps.
  - att+ln1 of subchunk c jump the job queue at the boundary (att's PSUM
    lives in a spool slot, not the FFN accumulator), so z1(c) is ready long
    before ffn(c) quarters drain.
  - FFN2(og) is ordered one og-slot behind FFN1(og+1) (h8 double-buffered)
    so the DVE relu latency hides under FFN1 matmuls.
  - w1 streams in og-column slices so weight arrival order matches FFN1's
    consumption order; subchunk input loads ride the Pool SWDGE queue so the
    bulk weight stream cannot delay them.
  - The softmax floor clamp is a single merged DVE op per step; the ctx
    normalize is a plain tensor_mul (the 8x ctx scale is folded into the 1/8
    ones row of vp on the host), split Pool/DVE by output partition half.

Precision: attention contributes only ~8.5% of the output (residual dominates)
so the whole attention path runs in fp8e4m3 (measured end-to-end rel err
~8.7e-3 vs the 2e-2 budget); the FFN runs in bf16 (fp8 FFN measured at
2.2-3.3e-2 - over budget). fp8 DoubleRow matmuls pair two 128-deep
contraction slices at 0.5 cycles/row.

Scaling ledger (to keep every fp8 tensor in e4m3's comfortable range):
  W*16 on host -> q',k',v' = 16x natural, scores s' = 256*s
  exp(s'/2048 - 6ln2) = 2^-6 * exp(s/8), clamped below at 2^-6 (the
  relu-softmax identity softmax(relu(s)) with exp(relu(x)) = max(exp(x),1))
  ctx8 = 8 * c[0:64]/c[64] = 128x natural (the 8x from the 1/8 ones row);
  att_psum = ctx8 @ (Wo.T*16) = 2048x natural -> y1 = att_psum/2048 + X.
"""

import os
import sys

for _p in ("/opt/trn_rl_repo",):
    if _p not in sys.path:
        sys.path.insert(0, _p)

_jp = os.environ.get("JAX_PLATFORMS")
if _jp is not None and "axon" not in _jp:
    os.environ["JAX_PLATFORMS"] = "axon," + _jp

import math

import ml_dtypes
import numpy as np

import concourse.bass as bass
import concourse.mybir as mybir
import concourse.tile as tile
from concourse import bacc
from concourse.bass_utils import run_bass_kernel_spmd

F32 = mybir.dt.float32
F8 = mybir.dt.float8e4
BF16 = mybir.dt.bfloat16
AF = mybir.ActivationFunctionType
OP = mybir.AluOpType
DRM = mybir.MatmulPerfMode.DoubleRow
E4 = ml_dtypes.float8_e4m3
BF = ml_dtypes.bfloat16

N_CORES = 8
B, S, D, H, DH, FF = 2, 2048, 1024, 16, 64, 4096
N = B * S            # 4096 tokens
P = 128
QC = N // N_CORES    # 512 tokens per core in phase B
NSUB = 4             # phase B token sub-chunks
QS = 128             # max subchunk size (PSUM tiles padded to this)
QSL = [128, 128, 128, 128]         # per-subchunk query counts (sum = QC)
OFF = [0, 128, 256, 384]           # per-subchunk token offsets
KI = S // P          # 16 key chunks per batch
EPS = 1e-5

WSC = 16.0                       # host weight scale for fp8
EXP_SCALE = 1.0 / 2048.0         # s' = 256*s ; s~ = s/8  -> 1/(8*256)
EXP_BIAS = float(-6.0 * math.log(2.0))   # probs scaled by 2^-6
PFLOOR = 2.0 ** -6
CTX_SCALE = 8.0                  # ctx8 = 8 * normalized ctx' (folded into the
#                                  1/8 ones row of vp on the host)
ATT_INV = 1.0 / 2048.0           # 1/(WSC*WSC*CTX_SCALE)

_CACHE = {}


# --------------------------------------------------------------------------
# Phase A: fp8 q/k/v projections for this core's 2 heads, all 4096 tokens.
#   xt8 [D, N]  X^T in fp8 (replicated)
#   wq8/wk8 [D, 128]  cols = [h0 dh0-31 | h1 dh0-31 | h0 dh32-63 | h1 dh32-63]
#   wv8 [D, 128]      cols = [h0 dh0-63 | h1 dh0-63]          (all x16)
# Outputs q8/k8 [128, N] rows in the same split-half order; v8 rows natural.
# --------------------------------------------------------------------------
def _build_phase_a():
    nc = bacc.Bacc("TRN2", target_bir_lowering=False, debug=False,
                   num_devices=N_CORES)
    xt8 = nc.dram_tensor("xt8", [D, N], F8, kind="ExternalInput")
    # host-pretiled [p, j, t, m] so the load is 1KB-contiguous per partition
    wq8 = nc.dram_tensor("wq8", [P, 4 * 2 * P], F8, kind="ExternalInput")
    wk8 = nc.dram_tensor("wk8", [P, 4 * 2 * P], F8, kind="ExternalInput")
    wv8 = nc.dram_tensor("wv8", [P, 4 * 2 * P], F8, kind="ExternalInput")
    q8 = nc.dram_tensor("q8", [P, N], F8, kind="ExternalOutput")
    k8 = nc.dram_tensor("k8", [P, N], F8, kind="ExternalOutput")
    v8 = nc.dram_tensor("v8", [P, N], F8, kind="ExternalOutput")
    CH = 512       # token columns per chunk (512B runs: full DMA rate)
    NO = N // CH   # 8 chunks

    with tile.TileContext(nc) as tc:
        with (
            tc.tile_pool(name="persist", bufs=1) as persist,
            tc.tile_pool(name="xp", bufs=3) as xp,
            tc.tile_pool(name="op", bufs=2) as op,
            tc.tile_pool(name="ps", bufs=2, space="PSUM") as ps,
        ):
            wq_sb = persist.tile([P, 4, 2, P], F8)
            wk_sb = persist.tile([P, 4, 2, P], F8)
            wv_sb = persist.tile([P, 4, 2, P], F8)
            for t_sb, t_dr in ((wq_sb, wq8), (wk_sb, wk8), (wv_sb, wv8)):
                nc.sync.dma_start(
                    t_sb[:],
                    t_dr.ap().rearrange("p (j t m) -> p j t m", j=4, t=2))

            state_o = []
            for o in range(NO):
                sl = slice(o * CH, (o + 1) * CH)
                xt_sb = xp.tile([P, 4, 2, CH], F8, name="xt")
                nc.sync.dma_start(
                    xt_sb[:],
                    xt8[:, sl].rearrange("(j t p) q -> p j t q", p=P, t=2))
                q_ps = ps.tile([P, CH], F32, name="q_ps")
                k_ps = ps.tile([P, CH], F32, name="k_ps")
                v_ps = ps.tile([P, CH], F32, name="v_ps")
                for w_sb, o_ps in ((wq_sb, q_ps), (wk_sb, k_ps),
                                   (wv_sb, v_ps)):
                    for j in range(4):
                        nc.tensor.matmul(o_ps[:], w_sb[:, j], xt_sb[:, j],
                                         start=(j == 0), stop=(j == 3),
                                         perf_mode=DRM)
                if o % 2 == 0:
                    outs = [op.tile([P, 2, CH], F8, name=nm)
                            for nm in ("qo", "ko", "vo")]
                    state_o.clear()
                    state_o.extend(outs)
                qo, ko, vo = state_o
                nc.vector.tensor_copy(qo[:, o % 2, :], q_ps[:])
                nc.scalar.activation(ko[:, o % 2, :], k_ps[:], AF.Copy)
                nc.vector.tensor_copy(vo[:, o % 2, :], v_ps[:])
                if o % 2 == 1:
                    # SWDGE queue: an out-DMA waiting on the copies must not
                    # head-of-line-block the xt input stream on the sync queue
                    sl2 = slice((o - 1) * CH, (o + 1) * CH)
                    nc.gpsimd.dma_start(q8[:, sl2], qo[:])
                    nc.gpsimd.dma_start(k8[:, sl2], ko[:])
                    nc.gpsimd.dma_start(v8[:, sl2], vo[:])
    nc.compile()
    return nc


# --------------------------------------------------------------------------
# Phase B: token-parallel attention + Wo + LN1 + FFN + LN2 for 512 tokens.
# Layouts (host-assembled, all partition-major 2D drams):
#   qt8 [128, 4*2*QC]   qt[32*bb+r, g, t, q] = q'[head 4g+bb, dh 32t+r, tok q]
#   kt8 [128, 4*2*S]    same over the batch's 2048 keys
#   vp8 [128, KI*H*65]  vp[p, kc, h, m<64] = v'[h, m, key 128kc+p]; [...,64]=1
#   wot8 [128, 4*2*D]   wot[p,j,t,m] = 16*Wo.T[64*(4j+2t+p//64)+p%64, m]
#   w1t [D, FF] bf16 (W1.T), w2t [FF, D] bf16 (W2.T), xts [D, QC] bf16 (X.T)
#   lnp [128, 40] f32: cols g1(8) be1(8) g2(8) be2(8) b2(8);  b1t [128,32] f32
# Output out_t [D, QC] f32.
# --------------------------------------------------------------------------
def _build_phase_b(trivial_affine):
    nc = bacc.Bacc("TRN2", target_bir_lowering=False, debug=False,
                   num_devices=N_CORES)
    qt8 = nc.dram_tensor("qt8", [P, 4 * 2 * QC], F8, kind="ExternalInput")
    kt8 = nc.dram_tensor("kt8", [P, 4 * 2 * S], F8, kind="ExternalInput")
    vp8 = nc.dram_tensor("vp8", [P, KI * H * 65], F8, kind="ExternalInput")
    wot8 = nc.dram_tensor("wot8", [P, 4 * 2 * D], F8, kind="ExternalInput")
    w1t = nc.dram_tensor("w1t", [D, FF], BF16, kind="ExternalInput")
    w2t = nc.dram_tensor("w2t", [FF, D], BF16, kind="ExternalInput")
    xts = nc.dram_tensor("xts", [D, QC], BF16, kind="ExternalInput")
    lnp = nc.dram_tensor("lnp", [P, 40], F32, kind="ExternalInput")
    b1t = nc.dram_tensor("b1t", [P, 32], F32, kind="ExternalInput")
    out_t = nc.dram_tensor("out_t", [D, QC], F32, kind="ExternalOutput")

    with tile.TileContext(nc) as tc:
        with (
            tc.tile_pool(name="persist", bufs=1) as persist,
            tc.tile_pool(name="xpool", bufs=2) as xpool,
            tc.tile_pool(name="zpool", bufs=2) as zpool,
            tc.tile_pool(name="hpool8", bufs=2) as hpool8,
            tc.tile_pool(name="slabp", bufs=3) as slabp,
            tc.tile_pool(name="ypool", bufs=2) as ypool,
            tc.tile_pool(name="sqp", bufs=1) as sqp,
            tc.tile_pool(name="craw", bufs=1) as craw,
            tc.tile_pool(name="ctxp", bufs=1) as ctxp,
            tc.tile_pool(name="outp", bufs=1) as outp,
            tc.tile_pool(name="bcp", bufs=2) as bcp,
            tc.tile_pool(name="bcb", bufs=1) as bcb,
            tc.tile_pool(name="smallp", bufs=1) as smallp,
            tc.tile_pool(name="spool", bufs=2, space="PSUM") as spool,
            tc.tile_pool(name="cpool", bufs=1, space="PSUM") as cpool,
            tc.tile_pool(name="fpool", bufs=1, space="PSUM") as fpool,
            tc.tile_pool(name="hpool", bufs=1, space="PSUM") as hpool,
        ):
            kt_sb = persist.tile([P, 4, 2, S], F8)

            vp_sb = persist.tile([P, KI, H, 65], F8)
            wot_sb = persist.tile([P, 4, 2, D], F8)
            w1_sb = persist.tile([P, 8, FF], BF16)
            w2_sb = persist.tile([P, 32, D], BF16)
            lnp_sb = persist.tile([P, 40], F32)
            b1_sb = persist.tile([P, 32], F32)
            ones_c = persist.tile([P, 1], BF16)
            ebias = persist.tile([P, 1], F32)
            epst = persist.tile([P, 1], F32)

            def load_persist():
                ktr = kt8.ap().rearrange("p (g t k) -> p g t k", g=4, t=2)
                vpr = vp8.ap().rearrange("p (kc h m) -> p kc h m",
                                         kc=KI, h=H)
                # kt g0 in halves: the first scores-half only needs the
                # first 8 key chunks, so the exp stream starts ~0.7us sooner
                nc.sync.dma_start(kt_sb[:, 0, :, 0:S // 2],
                                  ktr[:, 0, :, 0:S // 2])
                nc.sync.dma_start(kt_sb[:, 0, :, S // 2:S],
                                  ktr[:, 0, :, S // 2:S])
                # ctx(0,0) at ~10us contracts over ALL 16 vp key-chunks, so
                # the whole vp tensor must land before the later kt groups
                nc.sync.dma_start(vp_sb[:, 0:4], vpr[:, 0:4])
                nc.sync.dma_start(vp_sb[:, 4:8], vpr[:, 4:8])
                nc.sync.dma_start(vp_sb[:, 8:12], vpr[:, 8:12])
                nc.sync.dma_start(vp_sb[:, 12:KI], vpr[:, 12:KI])
                for g in range(1, 4):
                    nc.sync.dma_start(kt_sb[:, g], ktr[:, g])
                nc.sync.dma_start(lnp_sb[:], lnp.ap())
                nc.sync.dma_start(b1_sb[:], b1t.ap())
            nc.vector.memset(ones_c[:], 1.0)
            nc.vector.memset(ebias[:], EXP_BIAS)
            nc.vector.memset(epst[:], EPS)

            def w1_og(og):
                # w1 streams in og-column slices: FFN1(og) needs exactly one
                # slice (all kc rows of its 512 hidden columns), so arrival
                # order matches consumption order.
                nc.sync.dma_start(
                    w1_sb[:, :, og * 512:(og + 1) * 512],
                    w1t[:, og * 512:(og + 1) * 512].rearrange(
                        "(kc p) m -> p kc m", p=P))

            def w2_fk(j):
                nc.sync.dma_start(
                    w2_sb[:, 4 * j:4 * j + 4],
                    w2t[4 * j * P:(4 * j + 4) * P, :].rearrange(
                        "(f p) m -> p f m", p=P))

            def stream_weights():
                for og in range(4):
                    w1_og(og)
                w2_fk(0)
                nc.sync.dma_start(
                    wot_sb[:], wot8.ap().rearrange("p (j t m) -> p j t m",
                                                   j=4, t=2))
                for og in range(4, 8):
                    w1_og(og)
                    w2_fk(og - 3)
                for j in range(5, 8):
                    w2_fk(j)

            g1c = lambda kc: lnp_sb[:, kc:kc + 1]
            be1c = lambda kc: lnp_sb[:, 8 + kc:9 + kc]
            g2c = lambda kc: lnp_sb[:, 16 + kc:17 + kc]
            be2c = lambda kc: lnp_sb[:, 24 + kc:25 + kc]
            b2c = lambda kc: lnp_sb[:, 32 + kc:33 + kc]

            state = {}

            def scores_half(c, h, half):
                """scores matmuls + exp for one 8-key-chunk half of head h.
                PSUM zero-region rule: one start=True per 2KB bank region.
                After half 1, the relu-softmax floor clamp runs over the whole
                slab, alternating DVE/Pool per step to balance engine load."""
                qs = QSL[c]
                g, bb = h // 4, h % 4
                pp = slice(32 * bb, 32 * bb + 32)
                if half == 0:
                    state[c]["slab"][h] = slabp.tile([P, 16, qs], F8,
                                                     name="slab")
                slab = state[c]["slab"][h]
                s_ps = spool.tile([P, 8, QS], F32, name="s_ps")
                for kc8 in range(8):
                    kc = half * 8 + kc8
                    nc.tensor.matmul(
                        s_ps[:, kc8, 0:qs],
                        kt_sb[pp, g, :, kc * P:(kc + 1) * P],
                        state[c]["qt"][pp, g, :, :],
                        start=(kc8 % 4 == 0), stop=True,
                        perf_mode=DRM, skip_group_check=True,
                        tile_position=(32 * bb, 0))
                hsl = slice(8 * half, 8 * half + 8)
                nc.scalar.activation(slab[:, hsl, :], s_ps[:, :, 0:qs],
                                     AF.Exp, scale=EXP_SCALE, bias=ebias[:])
                if half == 1:
                    nc.vector.tensor_scalar_max(slab[:], slab[:], PFLOOR)

            def ctx_part(c, h):
                """ctx accumulation over keys + stage raw result to SBUF."""
                qs = QSL[c]
                slab = state[c]["slab"].pop(h)
                c_ps = cpool.tile([65, 512], F32, name="c_ps")
                for pj in range(8):
                    nc.tensor.matmul(
                        c_ps[:, 0:qs],
                        vp_sb[:, 2 * pj:2 * pj + 2, h, :],
                        slab[:, 2 * pj:2 * pj + 2, :],
                        start=(pj == 0), stop=(pj == 7),
                        perf_mode=DRM, skip_group_check=True)
                nc.vector.tensor_copy(state[c]["craw"][:, h, :],
                                      c_ps[:, 0:qs])

            def norm_group(c, grp):
                """Normalize heads 8*grp..8*grp+7 of subchunk c into ctx fp8.
                The ones row of vp is 1/8 on the host, so 1/craw[64] already
                carries the 8x ctx scale; normalization is a plain multiply.
                sig=0 keeps partitions aligned (Pool-safe); sig=1 shifts the
                output partition base, which only DVE handles."""
                qs = QSL[c]
                cr = state[c]["craw"]
                hs = slice(8 * grp, 8 * grp + 8)
                inv = bcb.tile([1, 8, qs], BF16, name="inv")
                with nc.allow_low_precision(reason="fp8 attention path"):
                    nc.vector.reciprocal(inv[:], cr[64:65, hs, :])
                invb = bcb.tile([64, 8, qs], BF16, name="invb")
                nc.gpsimd.partition_broadcast(invb[:], inv[:])
                ctx = state[c]["ctx"]
                for sig, eng in ((0, nc.gpsimd), (1, nc.vector)):
                    eng.tensor_mul(
                        ctx[64 * sig:64 * sig + 64, 4 * grp:4 * grp + 4, :],
                        cr[0:64, 8 * grp + sig:8 * grp + 8:2, :],
                        invb[:, sig::2, :])

            def rstd_via_lnexp(dst_rstd, var_ap):
                # rstd = exp(-0.5 * ln(var + eps)); stays in the exp/ln
                # ACT table set so no table reloads are ever needed.
                nc.scalar.activation(dst_rstd, var_ap, AF.Ln, bias=epst[0:1, :])
                nc.scalar.activation(dst_rstd, dst_rstd, AF.Exp, scale=-0.5)

            def layer_norm(y_sb, out_sb, gc, bec, qs):
                """t-layout LN over the 1024 partition-features.
                y_sb [P, 8, qs] bf16 -> out_sb (bf16 or f32)."""
                st = hpool.tile([P, 4, QS], F32, name="h_ps")
                # two accumulation chains on DIFFERENT partitions (0 and 32):
                # PSUM zero regions are per-partition, so both chains can use
                # clean start=True semantics (no lazy-zero assumptions).
                for kc in range(8):
                    nc.tensor.matmul(st[0:1, 0, 0:qs], ones_c[:],
                                     y_sb[:, kc, :],
                                     start=(kc == 0), stop=(kc == 7),
                                     skip_group_check=True)
                for hf in (0, 1):
                    sq = sqp.tile([P, 4, qs], BF16, name="sq")
                    nc.vector.tensor_mul(sq[:], y_sb[:, 4 * hf:4 * hf + 4, :],
                                         y_sb[:, 4 * hf:4 * hf + 4, :])
                    for k4 in range(4):
                        nc.tensor.matmul(st[32:33, 0, 0:qs], ones_c[:],
                                         sq[:, k4, :], start=(hf + k4 == 0),
                                         stop=(hf == 1 and k4 == 3),
                                         skip_group_check=True)
                stats = smallp.tile([1, 2, qs], F32, name="stats")
                stats_bf = smallp.tile([1, 2, qs], BF16, name="stats_bf")
                nc.vector.tensor_scalar_mul(stats[0:1, 0, :],
                                            st[0:1, 0, 0:qs], 1.0 / D)
                nc.vector.tensor_scalar_mul(stats[0:1, 1, :],
                                            st[32:33, 0, 0:qs], 1.0 / D)
                mean = stats[0:1, 0, :]
                nc.vector.tensor_mul(stats_bf[0:1, 0, :], mean, mean)
                nc.vector.tensor_sub(stats[0:1, 1, :], stats[0:1, 1, :],
                                     stats_bf[0:1, 0, :])       # var
                rstd_via_lnexp(stats[0:1, 1, :], stats[0:1, 1, :])
                nc.vector.tensor_mul(stats_bf[0:1, 1, :], mean,
                                     stats[0:1, 1, :])          # ms (bf16)
                nc.vector.tensor_copy(stats_bf[0:1, 0, :], stats[0:1, 1, :])
                for hf in (0, 1):
                    veng = nc.vector
                    rstd_b = bcb.tile([P, 4, qs], BF16, name="rstd_b")
                    ms_b = bcb.tile([P, 4, qs], BF16, name="ms_b")
                    for k4 in range(4):
                        nc.gpsimd.partition_broadcast(rstd_b[:, k4, :],
                                                      stats_bf[0:1, 0, :])
                        nc.gpsimd.partition_broadcast(ms_b[:, k4, :],
                                                      stats_bf[0:1, 1, :])
                    ks = slice(4 * hf, 4 * hf + 4)
                    tmp = sqp.tile([P, 4, qs], BF16, name="sq")
                    veng.tensor_mul(tmp[:], y_sb[:, ks, :], rstd_b[:])
                    if trivial_affine:
                        veng.tensor_sub(out_sb[:, ks, :], tmp[:], ms_b[:])
                    else:
                        nc.vector.tensor_sub(tmp[:], tmp[:], ms_b[:])
                        for k4 in range(4):
                            kc = 4 * hf + k4
                            nc.vector.tensor_scalar(
                                out=out_sb[:, kc, :], in0=tmp[:, k4, :],
                                scalar1=gc(kc), scalar2=bec(kc),
                                op0=OP.mult, op1=OP.add)

            def att_mm_y1(c):
                """Wo projection (fp8 DR) + residual -> y1[c] (bf16)."""
                qs = QSL[c]
                st = state[c]
                a_ps = spool.tile([P, 8, QS], F32, name="s_ps")
                for mc in range(8):
                    for j in range(4):
                        nc.tensor.matmul(
                            a_ps[:, mc, 0:qs],
                            wot_sb[:, j, :, mc * P:(mc + 1) * P],
                            st["ctx"][:, 2 * j:2 * j + 2, :],
                            start=(j == 0 and mc % 4 == 0), stop=(j == 3),
                            perf_mode=DRM, skip_group_check=True)
                y1 = ypool.tile([P, 8, qs], BF16, name="y1")
                nc.vector.scalar_tensor_tensor(
                    out=y1[:], in0=a_ps[:, :, 0:qs], scalar=ATT_INV,
                    in1=st["xts"][:], op0=OP.mult, op1=OP.add)
                st["y1"] = y1

            def ln1(c):
                """LN1 without rstd: with trivial affine and b1=0, the FFN is
                positively homogeneous per token (relu(a*x)=a*relu(x), a>0),
                so y2 = f + z1 = rstd1*(f' + z1') and LN2 cancels the rstd1
                exactly (eps shift ~1e-6 relative). Only the mean is removed.
                Falls back to the full LN when affine params are nontrivial."""
                qs = QSL[c]
                st = state[c]
                z1 = zpool.tile([P, 8, qs], BF16, name="z1")
                if not trivial_affine:
                    layer_norm(st["y1"], z1, g1c, be1c, qs)
                    st["z1"] = z1
                    return
                y = st["y1"]
                stq = hpool.tile([P, 4, QS], F32, name="h_ps")
                for kc in range(8):
                    nc.tensor.matmul(stq[0:1, 0, 0:qs], ones_c[:], y[:, kc, :],
                                     start=(kc == 0), stop=(kc == 7),
                                     skip_group_check=True)
                mu_bf = smallp.tile([1, 1, qs], BF16, name="mu_bf")
                nc.vector.tensor_scalar_mul(mu_bf[0:1, 0, :],
                                            stq[0:1, 0, 0:qs], 1.0 / D)
                mu_b = bcb.tile([P, 4, qs], BF16, name="mu_b")
                for k4 in range(4):
                    nc.gpsimd.partition_broadcast(mu_b[:, k4, :],
                                                  mu_bf[0:1, 0, :])
                for hf in (0, 1):
                    nc.vector.tensor_sub(z1[:, 4 * hf:4 * hf + 4, :],
                                         y[:, 4 * hf:4 * hf + 4, :], mu_b[:])
                st["z1"] = z1

            def ffn_quarter(c, qi):
                """FFN work in 16-matmul quarters: og = qi//4,
                quarter 0/1 = FFN1 halves (+relu), 2/3 = FFN2 halves.
                The caller orders FFN2(og) one og-slot behind FFN1(og+1) so
                the DVE relu latency hides under FFN1 matmuls; h8 is
                double-buffered per og to allow it."""
                qs = QSL[c]
                st = state[c]
                og, qq = qi // 4, qi % 4
                if qq in (0, 1):
                    if qq == 0:
                        st["h_ps"] = hpool.tile([P, 4, QS], F32, name="h_ps")
                    h_ps = st["h_ps"]
                    for mc4 in (2 * qq, 2 * qq + 1):
                        for kc in range(8):
                            nc.tensor.matmul(
                                h_ps[:, mc4, 0:qs],
                                w1_sb[:, kc, og * 512 + mc4 * P:
                                      og * 512 + (mc4 + 1) * P],
                                st["z1"][:, kc, :],
                                start=(mc4 == 0 and kc == 0),
                                stop=(kc == 7), skip_group_check=True)
                    if qq == 1:
                        h8 = hpool8.tile([P, 4, qs], BF16, name="h8")
                        if trivial_affine:
                            nc.vector.tensor_scalar_max(h8[:],
                                                        h_ps[:, :, 0:qs], 0.0)
                        else:
                            for mc4 in range(4):
                                nc.scalar.activation(
                                    h8[:, mc4, :], h_ps[:, mc4, 0:qs],
                                    AF.Relu,
                                    bias=b1_sb[:, og * 4 + mc4:
                                               og * 4 + mc4 + 1])
                        st["h8"][og] = h8
                else:
                    if og == 0 and qq == 2:
                        st["f_ps"] = fpool.tile([P, 8, QS], F32, name="f_ps",
                                                tag="f_ps")
                    f_ps = st["f_ps"]
                    for i in (2 * (qq - 2), 2 * (qq - 2) + 1):
                        fk = og * 4 + i
                        for mc in range(8):
                            nc.tensor.matmul(
                                f_ps[:, mc, 0:qs],
                                w2_sb[:, fk, mc * P:(mc + 1) * P],
                                st["h8"][og][:, i, :],
                                start=(fk == 0 and mc % 4 == 0),
                                stop=(fk == 31), skip_group_check=True)
                    if qq == 3:
                        del st["h8"][og]

            def finish_a(c):
                """y2 = f + (b2) + z1 and LN2 stats through var - no ACT op.
                The ACT rstd goes in finish_b, enqueued a job later, so it
                never head-of-line-blocks the exp stream while var is still
                in flight."""
                qs = QSL[c]
                st = state[c]
                y2 = ypool.tile([P, 8, qs], BF16, name="y1")
                if trivial_affine:
                    nc.vector.tensor_add(y2[:], st["f_ps"][:, :, 0:qs],
                                         st["z1"][:])
                else:
                    for mc in range(8):
                        nc.vector.scalar_tensor_tensor(
                            out=y2[:, mc, :], in0=st["f_ps"][:, mc, 0:qs],
                            scalar=b2c(mc), in1=st["z1"][:, mc, :],
                            op0=OP.add, op1=OP.add)
                st["y2"] = y2
                stq = hpool.tile([P, 4, QS], F32, name="h_ps")
                for kc in range(8):
                    nc.tensor.matmul(stq[0:1, 0, 0:qs], ones_c[:],
                                     y2[:, kc, :],
                                     start=(kc == 0), stop=(kc == 7),
                                     skip_group_check=True)
                for hf in (0, 1):
                    sq = sqp.tile([P, 4, qs], BF16, name="sq")
                    nc.vector.tensor_mul(sq[:], y2[:, 4 * hf:4 * hf + 4, :],
                                         y2[:, 4 * hf:4 * hf + 4, :])
                    for k4 in range(4):
                        nc.tensor.matmul(stq[32:33, 0, 0:qs], ones_c[:],
                                         sq[:, k4, :], start=(hf + k4 == 0),
                                         stop=(hf == 1 and k4 == 3),
                                         skip_group_check=True)
                stats = smallp.tile([1, 2, qs], F32, name="stats")
                stats_bf = smallp.tile([1, 2, qs], BF16, name="stats_bf")
                nc.vector.tensor_scalar_mul(stats[0:1, 0, :],
                                            stq[0:1, 0, 0:qs], 1.0 / D)
                nc.vector.tensor_scalar_mul(stats[0:1, 1, :],
                                            stq[32:33, 0, 0:qs], 1.0 / D)
                mean = stats[0:1, 0, :]
                nc.vector.tensor_mul(stats_bf[0:1, 0, :], mean, mean)
                nc.vector.tensor_sub(stats[0:1, 1, :], stats[0:1, 1, :],
                                     stats_bf[0:1, 0, :])       # var
                st["ln2"] = (stats, stats_bf)

            def finish_b(c):
                """rstd (ACT), normalize, write out subchunk c."""
                qs = QSL[c]
                st = state[c]
                y2 = st["y2"]
                stats, stats_bf = st["ln2"]
                mean = stats[0:1, 0, :]
                rstd_via_lnexp(stats[0:1, 1, :], stats[0:1, 1, :])
                nc.vector.tensor_mul(stats_bf[0:1, 1, :], mean,
                                     stats[0:1, 1, :])          # ms (bf16)
                nc.vector.tensor_copy(stats_bf[0:1, 0, :], stats[0:1, 1, :])
                for hf in (0, 1):
                    rstd_b = bcb.tile([P, 4, qs], BF16, name="rstd_b")
                    ms_b = bcb.tile([P, 4, qs], BF16, name="ms_b")
                    for k4 in range(4):
                        nc.gpsimd.partition_broadcast(rstd_b[:, k4, :],
                                                      stats_bf[0:1, 0, :])
                        nc.gpsimd.partition_broadcast(ms_b[:, k4, :],
                                                      stats_bf[0:1, 1, :])
                    ks = slice(4 * hf, 4 * hf + 4)
                    z2 = outp.tile([P, 4, qs], F32, name="z2")
                    tmp = sqp.tile([P, 4, qs], BF16, name="sq")
                    nc.vector.tensor_mul(tmp[:], y2[:, ks, :], rstd_b[:])
                    if trivial_affine:
                        nc.vector.tensor_sub(z2[:], tmp[:], ms_b[:])
                    else:
                        nc.vector.tensor_sub(tmp[:], tmp[:], ms_b[:])
                        for k4 in range(4):
                            kc = 4 * hf + k4
                            nc.vector.tensor_scalar(
                                out=z2[:, k4, :], in0=tmp[:, k4, :],
                                scalar1=g2c(kc), scalar2=be2c(kc),
                                op0=OP.mult, op1=OP.add)
                    nc.sync.dma_start(
                        out_t[4 * hf * P:4 * hf * P + 4 * P,
                              OFF[c]:OFF[c] + qs].rearrange(
                            "(kc p) q -> p kc q", p=P), z2[:])
                del state[c]

            # ---- global software-pipelined stream over 64 head-steps ----
            # PE emission per step: ctx(step-3) | scores half 0 | ~1.2us of
            # queued jobs | scores half 1 | rest of the job quantum. The
            # scores halves single-buffer through one 2-bank PSUM slot; the
            # job quantum between them covers the exp WAR latency. Boundary
            # work (norms, Wo+LN1, FFN quarters, LN2+out) drains from the job
            # queue paced by estimated PE-ns so the exp stream never starves.
            jobs = []

            def drain(budget_ns, keep_min=0):
                spent, n = 0, 0
                while len(jobs) > keep_min and spent < budget_ns and n < 8:
                    cost, fn = jobs.pop(0)
                    fn()
                    spent += cost
                    n += 1

            def load_inputs(c, eng=None):
                # later subchunks load via the Pool SWDGE queue so they are
                # not stuck behind the bulk w1/w2 stream on the sync queue
                qs = QSL[c]
                eng = eng or nc.sync
                xts_sb = xpool.tile([P, 8, qs], BF16, name="xts")
                eng.dma_start(
                    xts_sb[:],
                    xts[:, OFF[c]:OFF[c] + qs].rearrange(
                        "(kc p) q -> p kc q", p=P))
                qt_c = xpool.tile([P, 4, 2, qs], F8, name="qt")
                eng.dma_start(
                    qt_c[:],
                    qt8.ap().rearrange("p (g t q) -> p g t q", g=4, t=2)[
                        :, :, :, OFF[c]:OFF[c] + qs])
                state[c] = {
                    "qt": qt_c,
                    "ctx": ctxp.tile([P, 8, qs], F8, name="ctx"),
                    "craw": craw.tile([65, H, qs], BF16, name="craw"),
                    "xts": xts_sb,
                    "slab": {},
                    "h8": {},
                }

            def enqueue_ffn(cp):
                # FFN2(og) trails FFN1(og+1) so relu latency hides
                order = []
                for og in range(8):
                    order += [4 * og, 4 * og + 1]
                    if og >= 1:
                        order += [4 * (og - 1) + 2, 4 * (og - 1) + 3]
                order += [30, 31]
                jc = int(860 * QSL[cp] / 128)
                for n_q, qi in enumerate(order):
                    jobs.append(
                        (jc, lambda cp=cp, qi=qi: ffn_quarter(cp, qi)))
                    if n_q == 1 and cp >= 1:
                        # rstd of LN2(cp-1): two quarters after finish_a so
                        # var is computed before the Ln lands on ACT
                        jobs.append((200, lambda cp=cp: finish_b(cp - 1)))
                jobs.append((jc, lambda cp=cp: finish_a(cp)))

            load_inputs(0)
            load_persist()
            load_inputs(1)
            stream_weights()

            NSTEP = NSUB * H
            for s in range(NSTEP + 3):
                if s >= 3:
                    cp, hp = divmod(s - 3, H)
                    ctx_part(cp, hp)
                    if hp == 7:
                        jobs.insert(0, (0, lambda cp=cp: norm_group(cp, 0)))
                    elif hp == 15:
                        norm_group(cp, 1)
                        # att+ln1 jump the queue so z1(cp) is ready before
                        # ffn(cp) quarters drain; att's a_ps lives in a spool
                        # slot so it does not wait on finish_a(cp-1)'s f_ps
                        jobs.insert(0, (900, lambda cp=cp: att_mm_y1(cp)))
                        jobs.insert(min(3, len(jobs)),
                                    (900, lambda cp=cp: ln1(cp)))
                        enqueue_ffn(cp)
                if s < NSTEP:
                    c, h = divmod(s, H)
                    drain(1600)
                    scores_half(c, h, 0)
                    scores_half(c, h, 1)
                    if h == 15 and c + 2 < NSUB:
                        load_inputs(c + 2, eng=nc.gpsimd)
                else:
                    drain(2400)
            while jobs:
                drain(10**9)
            finish_b(NSUB - 1)
    _pin_act_tables(nc)
    nc.compile()
    return nc


def _pin_act_tables(nc):
    """Restrict the act-table-load pass to the one set that covers every
    ACT function this kernel uses (exp, ln, relu, copy), so exactly one
    table load is emitted instead of per-transition reloads."""
    import bass_rust as _bass_rust
    from concourse.hw_specs import get_activation_tables
    tabs = get_activation_tables(nc.m.arch)
    # keep every entry (act_func_set_id is the index into act_info.json's
    # list) but empty all other sets so the pass can only pick this one.
    only = [(k, (v if k == "natural_log_exp_and_others" else set()))
            for k, v in tabs.items()]
    if any(v for _, v in only):
        nc.insert_act_table_loads = (
            lambda: _bass_rust.insert_act_table_loads(nc, only))


def _get(name, builder, *args):
    if name not in _CACHE:
        _CACHE[name] = builder(*args)
    return _CACHE[name]


def _qcols(c):
    h0, h1 = 2 * c, 2 * c + 1
    r = np.arange(32)
    return np.concatenate([h0 * 64 + r, h1 * 64 + r,
                           h0 * 64 + 32 + r, h1 * 64 + 32 + r])


def _vcols(c):
    h0, h1 = 2 * c, 2 * c + 1
    r = np.arange(64)
    return np.concatenate([h0 * 64 + r, h1 * 64 + r])


def kernel(X, Wq, Wk, Wo, ln1_g, ln1_b, ln2_g, ln2_b, W1, b1, W2, b2):
    f32 = lambda a: np.asarray(a, np.float32)
    X, Wq, Wk, Wo, W1, W2 = map(f32, (X, Wq, Wk, Wo, W1, W2))
    ln1_g, ln1_b, ln2_g, ln2_b, b1, b2 = map(
        f32, (ln1_g, ln1_b, ln2_g, ln2_b, b1, b2))
    Xf = X.reshape(N, D)
    Xt8 = np.ascontiguousarray(Xf.T).astype(E4)          # [D, N]
    WqT, WkT, WoT = Wq.T, Wk.T, Wo.T

    # ---------------- phase A ----------------
    nc_a = _get("a", _build_phase_a)

    def _pretile(arr):   # [D, P] -> [p, (j t m)] with row (j*2+t)*128+p
        return np.ascontiguousarray(
            arr.reshape(4, 2, P, P).transpose(2, 0, 1, 3).reshape(P, 8 * P))

    in_a = []
    for c in range(N_CORES):
        qc, vc = _qcols(c), _vcols(c)
        in_a.append({
            "xt8": Xt8,
            "wq8": _pretile((WqT[:, qc] * WSC).astype(E4)),
            "wk8": _pretile((WkT[:, qc] * WSC).astype(E4)),
            "wv8": _pretile((WoT[:, vc] * WSC).astype(E4)),
        })
    res_a = run_bass_kernel_spmd(nc_a, in_a, core_ids=list(range(N_CORES)))

    # assemble full q/k/v byte arrays [H, DH, N] (uint8 views of e4m3)
    u8 = np.uint8
    qfull = np.empty((H, DH, N), u8)
    kfull = np.empty((H, DH, N), u8)
    vfull = np.empty((H, DH, N), u8)
    for c in range(N_CORES):
        r = res_a.results[c]
        qa = np.asarray(r["q8"]).view(u8).reshape(2, 2, 32, N)  # [i, j, r, :]
        ka = np.asarray(r["k8"]).view(u8).reshape(2, 2, 32, N)
        va = np.asarray(r["v8"]).view(u8).reshape(2, DH, N)     # [j, dh, :]
        for j in range(2):
            qfull[2 * c + j] = qa[:, j].reshape(DH, N)
            kfull[2 * c + j] = ka[:, j].reshape(DH, N)
            vfull[2 * c + j] = va[j]

    # ---------------- phase B host layouts ----------------
    trivial = (not b1.any()) and (not b2.any()) and \
        np.all(ln1_g == 1) and (not ln1_b.any()) and \
        np.all(ln2_g == 1) and (not ln2_b.any())
    nc_b = _get("b", _build_phase_b, trivial)

    w1t = np.ascontiguousarray(W1.astype(BF).T)          # [D, FF] bf16
    w2t = np.ascontiguousarray(W2.astype(BF).T)          # [FF, D] bf16
    one8 = np.float32(1.0 / CTX_SCALE).astype(E4).view(u8)  # ones row = 1/8
    wot_arr = (WoT * WSC).astype(E4).view(u8)            # [D, D]
    # wot8[p, j, t, m] = WoT*16[64*(4j+2t+p//64)+p%64, m]
    wot8 = wot_arr.reshape(4, 2, 2, 64, D).transpose(2, 3, 0, 1, 4) \
        .reshape(P, 4 * 2 * D).copy()
    lnp = np.zeros((P, 40), np.float32)
    for i, vvec in enumerate((ln1_g, ln1_b, ln2_g, ln2_b, b2)):
        lnp[:, 8 * i:8 * i + 8] = vvec.reshape(8, P).T
    b1t = np.ascontiguousarray(b1.reshape(32, P).T)

    in_b = []
    for c in range(N_CORES):
        toks = slice(c * QC, (c + 1) * QC)
        bb = (c * QC) // S
        keys = slice(bb * S, (bb + 1) * S)
        # qt8 [128, 4, 2, QC]: [bb*32+r, g, t, q] = qfull[4g+bb, 32t+r, tok]
        qx = qfull[:, :, toks].reshape(4, 4, 2, 32, QC)   # [g, bb, t, r, q]
        qt8 = qx.transpose(1, 3, 0, 2, 4).reshape(P, 4 * 2 * QC).copy()
        kx = kfull[:, :, keys].reshape(4, 4, 2, 32, S)
        kt8 = kx.transpose(1, 3, 0, 2, 4).reshape(P, 4 * 2 * S).copy()
        # vp8 [128, kc, h, 65]
        vv = vfull[:, :, keys].reshape(H, DH, KI, P).transpose(3, 2, 0, 1)
        vp8 = np.concatenate(
            [vv, np.full((P, KI, H, 1), one8, u8)], axis=3) \
            .reshape(P, KI * H * 65).copy()
        in_b.append({
            "qt8": qt8.view(E4), "kt8": kt8.view(E4), "vp8": vp8.view(E4),
            "wot8": wot8.view(E4),
            "w1t": w1t, "w2t": w2t,
            "xts": np.ascontiguousarray(Xf.T[:, toks]).astype(BF),
            "lnp": lnp, "b1t": b1t,
        })
    res_b = run_bass_kernel_spmd(nc_b, in_b, core_ids=list(range(N_CORES)))
    out_t = np.concatenate(
        [np.asarray(res_b.results[c]["out_t"]) for c in range(N_CORES)],
        axis=1)                                          # [D, N]
    return np.ascontiguousarray(out_t.T).reshape(B, S, D).astype(np.float32)



# revision 86
# speedup vs baseline: 1.0092x; 1.0064x over previous
"""Trainium2 Bass kernel for nn_Encoder (dense transformer block), 8 NeuronCores.

Two-phase plan built around hiding the ACT-engine exp pass (the irreducible
~133 us/core softmax cost) underneath the FFN matmuls:

  Phase A (head-parallel, tiny): core c computes q,k,v projections for its
    2 heads over all 4096 tokens in fp8 DoubleRow matmuls (~27 us).
  Host: pure byte-level reshuffles of the fp8 q/k/v into the layouts phase B
    wants (no arithmetic).
  Phase B (token-parallel): core c runs full attention (all 16 heads) for its
    512 query tokens + Wo proj + LN1 + FFN + LN2. Tokens are processed in 4
    sub-chunks of 128, software-pipelined so subchunk c's scores/exp/ctx
    (PE+ACT+DVE) overlap subchunk c-1's FFN matmuls (PE) - exp runs in the
    shadow of the FFN.

Scheduling notes (what the ~100us over the baseline came from):
  - The exp stream is sacred: every ACT op other than exp is either removed
    (LN1 needs no rstd: with trivial affine+b1=0 the FFN is positively
    homogeneous per token, so y2 = rstd1*(f'+z1') and LN2 cancels rstd1
    exactly) or emitted only after its inputs are compute-complete (LN2's
    rstd rides in finish_b, two jobs after finish_a computed the variance),
    because the strict-FIFO ACT queue otherwise head-of-line-blocks exps.
  - att+ln1 of subchunk c jump the job queue at the boundary (att's PSUM
    lives in a spool slot, not the FFN accumulator), so z1(c) is ready long
    before ffn(c) quarters drain.
  - FFN2(og) is ordered one og-slot behind FFN1(og+1) (h8 double-buffered)
    so the DVE relu latency hides under FFN1 matmuls.
  - w1 streams in og-column slices so weight arrival order matches FFN1's
    consumption order; subchunk input loads ride the Pool SWDGE queue so the
    bulk weight stream cannot delay them.
  - The softmax floor clamp is a single merged DVE op per step; the ctx
    normalize is a plain tensor_mul (the 8x ctx scale is folded into the 1/8
    ones row of vp on the host), split Pool/DVE by output partition half.

Precision: attention contributes only ~8.5% of the output (residual dominates)
so the whole attention path runs in fp8e4m3 (measured end-to-end rel err
~8.7e-3 vs the 2e-2 budget); the FFN runs in bf16 (fp8 FFN measured at
2.2-3.3e-2 - over budget). fp8 DoubleRow matmuls pair two 128-deep
contraction slices at 0.5 cycles/row.

Scaling ledger (to keep every fp8 tensor in e4m3's comfortable range):
  W*16 on host -> q',k',v' = 16x natural, scores s' = 256*s
  exp(s'/2048 - 6ln2) = 2^-6 * exp(s/8), clamped below at 2^-6 (the
  relu-softmax identity softmax(relu(s)) with exp(relu(x)) = max(exp(x),1))
  ctx8 = 8 * c[0:64]/c[64] = 128x natural (the 8x from the 1/8 ones row);
  att_psum = ctx8 @ (Wo.T*16) = 2048x natural -> y1 = att_psum/2048 + X.
"""

import os
import sys

for _p in ("/opt/trn_rl_repo",):
    if _p not in sys.path:
        sys.path.insert(0, _p)

_jp = os.environ.get("JAX_PLATFORMS")
if _jp is not None and "axon" not in _jp:
    os.environ["JAX_PLATFORMS"] = "axon," + _jp

import math

import ml_dtypes
import numpy as np

import concourse.bass as bass
import concourse.mybir as mybir
import concourse.tile as tile
from concourse import bacc
from concourse.bass_utils import run_bass_kernel_spmd

F32 = mybir.dt.float32
F8 = mybir.dt.float8e4
BF16 = mybir.dt.bfloat16
AF = mybir.ActivationFunctionType
OP = mybir.AluOpType
DRM = mybir.MatmulPerfMode.DoubleRow
E4 = ml_dtypes.float8_e4m3
BF = ml_dtypes.bfloat16

N_CORES = 8
B, S, D, H, DH, FF = 2, 2048, 1024, 16, 64, 4096
N = B * S            # 4096 tokens
P = 128
QC = N // N_CORES    # 512 tokens per core in phase B
NSUB = 4             # phase B token sub-chunks
QS = 128             # max subchunk size (PSUM tiles padded to this)
QSL = [128, 128, 128, 128]         # per-subchunk query counts (sum = QC)
OFF = [0, 128, 256, 384]           # per-subchunk token offsets
KI = S // P          # 16 key chunks per batch
EPS = 1e-5

WSC = 16.0                       # host weight scale for fp8
EXP_SCALE = 1.0 / 2048.0         # s' = 256*s ; s~ = s/8  -> 1/(8*256)
EXP_BIAS = float(-6.0 * math.log(2.0))   # probs scaled by 2^-6
PFLOOR = 2.0 ** -6
CTX_SCALE = 8.0                  # ctx8 = 8 * normalized ctx' (folded into the
#                                  1/8 ones row of vp on the host)
ATT_INV = 1.0 / 2048.0           # 1/(WSC*WSC*CTX_SCALE)

_CACHE = {}


# --------------------------------------------------------------------------
# Phase A: fp8 q/k/v projections for this core's 2 heads, all 4096 tokens.
#   xt8 [D, N]  X^T in fp8 (replicated)
#   wq8/wk8 [D, 128]  cols = [h0 dh0-31 | h1 dh0-31 | h0 dh32-63 | h1 dh32-63]
#   wv8 [D, 128]      cols = [h0 dh0-63 | h1 dh0-63]          (all x16)
# Outputs q8/k8 [128, N] rows in the same split-half order; v8 rows natural.
# --------------------------------------------------------------------------
def _build_phase_a():
    nc = bacc.Bacc("TRN2", target_bir_lowering=False, debug=False,
                   num_devices=N_CORES)
    xt8 = nc.dram_tensor("xt8", [D, N], F8, kind="ExternalInput")
    # host-pretiled [p, j, t, m] so the load is 1KB-contiguous per partition
    wq8 = nc.dram_tensor("wq8", [P, 4 * 2 * P], F8, kind="ExternalInput")
    wk8 = nc.dram_tensor("wk8", [P, 4 * 2 * P], F8, kind="ExternalInput")
    wv8 = nc.dram_tensor("wv8", [P, 4 * 2 * P], F8, kind="ExternalInput")
    q8 = nc.dram_tensor("q8", [P, N], F8, kind="ExternalOutput")
    k8 = nc.dram_tensor("k8", [P, N], F8, kind="ExternalOutput")
    v8 = nc.dram_tensor("v8", [P, N], F8, kind="ExternalOutput")
    CH = 512       # token columns per chunk (512B runs: full DMA rate)
    NO = N // CH   # 8 chunks

    with tile.TileContext(nc) as tc:
        with (
            tc.tile_pool(name="persist", bufs=1) as persist,
            tc.tile_pool(name="xp", bufs=3) as xp,
            tc.tile_pool(name="op", bufs=2) as op,
            tc.tile_pool(name="ps", bufs=2, space="PSUM") as ps,
        ):
            wq_sb = persist.tile([P, 4, 2, P], F8)
            wk_sb = persist.tile([P, 4, 2, P], F8)
            wv_sb = persist.tile([P, 4, 2, P], F8)
            for t_sb, t_dr in ((wq_sb, wq8), (wk_sb, wk8), (wv_sb, wv8)):
                nc.sync.dma_start(
                    t_sb[:],
                    t_dr.ap().rearrange("p (j t m) -> p j t m", j=4, t=2))

            state_o = []
            for o in range(NO):
                sl = slice(o * CH, (o + 1) * CH)
                xt_sb = xp.tile([P, 4, 2, CH], F8, name="xt")
                nc.gpsimd.dma_start(
                    xt_sb[:],
                    xt8[:, sl].rearrange("(j t p) q -> p j t q", p=P, t=2))
                q_ps = ps.tile([P, CH], F32, name="q_ps")
                k_ps = ps.tile([P, CH], F32, name="k_ps")
                v_ps = ps.tile([P, CH], F32, name="v_ps")
                for w_sb, o_ps in ((wq_sb, q_ps), (wk_sb, k_ps),
                                   (wv_sb, v_ps)):
                    for j in range(4):
                        nc.tensor.matmul(o_ps[:], w_sb[:, j], xt_sb[:, j],
                                         start=(j == 0), stop=(j == 3),
                                         perf_mode=DRM)
                if o % 2 == 0:
                    outs = [op.tile([P, 2, CH], F8, name=nm)
                            for nm in ("qo", "ko", "vo")]
                    state_o.clear()
                    state_o.extend(outs)
                qo, ko, vo = state_o
                nc.vector.tensor_copy(qo[:, o % 2, :], q_ps[:])
                nc.scalar.activation(ko[:, o % 2, :], k_ps[:], AF.Copy)
                nc.vector.tensor_copy(vo[:, o % 2, :], v_ps[:])
                if o % 2 == 1:
                    # outs ride sync/HWDGE; the xt ins ride the Pool SWDGE
                    # queue so neither stream head-of-line-blocks the other
                    # (and HWDGE descriptor-gen is ~2x faster than Pool's)
                    sl2 = slice((o - 1) * CH, (o + 1) * CH)
                    nc.sync.dma_start(q8[:, sl2], qo[:])
                    nc.sync.dma_start(k8[:, sl2], ko[:])
                    nc.sync.dma_start(v8[:, sl2], vo[:])
    nc.compile()
    return nc


# --------------------------------------------------------------------------
# Phase B: token-parallel attention + Wo + LN1 + FFN + LN2 for 512 tokens.
# Layouts (host-assembled, all partition-major 2D drams):
#   qt8 [128, 4*2*QC]   qt[32*bb+r, g, t, q] = q'[head 4g+bb, dh 32t+r, tok q]
#   kt8 [128, 4*2*S]    same over the batch's 2048 keys
#   vp8 [128, KI*H*65]  vp[p, kc, h, m<64] = v'[h, m, key 128kc+p]; [...,64]=1
#   wot8 [128, 4*2*D]   wot[p,j,t,m] = 16*Wo.T[64*(4j+2t+p//64)+p%64, m]
#   w1t [D, FF] bf16 (W1.T), w2t [FF, D] bf16 (W2.T), xts [D, QC] bf16 (X.T)
#   lnp [128, 40] f32: cols g1(8) be1(8) g2(8) be2(8) b2(8);  b1t [128,32] f32
# Output out_t [D, QC] f32.
# --------------------------------------------------------------------------
def _build_phase_b(trivial_affine):
    nc = bacc.Bacc("TRN2", target_bir_lowering=False, debug=False,
                   num_devices=N_CORES)
    qt8 = nc.dram_tensor("qt8", [P, 4 * 2 * QC], F8, kind="ExternalInput")
    kt8 = nc.dram_tensor("kt8", [P, 4 * 2 * S], F8, kind="ExternalInput")
    vp8 = nc.dram_tensor("vp8", [P, KI * H * 65], F8, kind="ExternalInput")
    wot8 = nc.dram_tensor("wot8", [P, 4 * 2 * D], F8, kind="ExternalInput")
    w1t = nc.dram_tensor("w1t", [D, FF], BF16, kind="ExternalInput")
    w2t = nc.dram_tensor("w2t", [FF, D], BF16, kind="ExternalInput")
    xts = nc.dram_tensor("xts", [D, QC], BF16, kind="ExternalInput")
    lnp = nc.dram_tensor("lnp", [P, 40], F32, kind="ExternalInput")
    b1t = nc.dram_tensor("b1t", [P, 32], F32, kind="ExternalInput")
    out_t = nc.dram_tensor("out_t", [D, QC], F32, kind="ExternalOutput")

    with tile.TileContext(nc) as tc:
        with (
            tc.tile_pool(name="persist", bufs=1) as persist,
            tc.tile_pool(name="xpool", bufs=2) as xpool,
            tc.tile_pool(name="zpool", bufs=2) as zpool,
            tc.tile_pool(name="hpool8", bufs=2) as hpool8,
            tc.tile_pool(name="slabp", bufs=3) as slabp,
            tc.tile_pool(name="ypool", bufs=2) as ypool,
            tc.tile_pool(name="sqp", bufs=1) as sqp,
            tc.tile_pool(name="craw", bufs=1) as craw,
            tc.tile_pool(name="ctxp", bufs=1) as ctxp,
            tc.tile_pool(name="outp", bufs=1) as outp,
            tc.tile_pool(name="bcp", bufs=2) as bcp,
            tc.tile_pool(name="bcb", bufs=1) as bcb,
            tc.tile_pool(name="smallp", bufs=1) as smallp,
            tc.tile_pool(name="spool", bufs=2, space="PSUM") as spool,
            tc.tile_pool(name="cpool", bufs=1, space="PSUM") as cpool,
            tc.tile_pool(name="fpool", bufs=1, space="PSUM") as fpool,
            tc.tile_pool(name="hpool", bufs=1, space="PSUM") as hpool,
        ):
            kt_sb = persist.tile([P, 4, 2, S], F8)

            vp_sb = persist.tile([P, KI, H, 65], F8)
            wot_sb = persist.tile([P, 4, 2, D], F8)
            w1_sb = persist.tile([P, 8, FF], BF16)
            w2_sb = persist.tile([P, 32, D], BF16)
            lnp_sb = persist.tile([P, 40], F32)
            b1_sb = persist.tile([P, 32], F32)
            ones_c = persist.tile([P, 1], BF16)
            ebias = persist.tile([P, 1], F32)
            epst = persist.tile([P, 1], F32)

            def load_persist():
                ktr = kt8.ap().rearrange("p (g t k) -> p g t k", g=4, t=2)
                vpr = vp8.ap().rearrange("p (kc h m) -> p kc h m",
                                         kc=KI, h=H)
                # kt g0 in halves: the first scores-half only needs the
                # first 8 key chunks, so the exp stream starts ~0.7us sooner
                nc.sync.dma_start(kt_sb[:, 0, :, 0:S // 2],
                                  ktr[:, 0, :, 0:S // 2])
                nc.sync.dma_start(kt_sb[:, 0, :, S // 2:S],
                                  ktr[:, 0, :, S // 2:S])
                # ctx(0,0) at ~10us contracts over ALL 16 vp key-chunks, so
                # the whole vp tensor must land before the later kt groups
                nc.sync.dma_start(vp_sb[:, 0:4], vpr[:, 0:4])
                nc.sync.dma_start(vp_sb[:, 4:8], vpr[:, 4:8])
                nc.sync.dma_start(vp_sb[:, 8:12], vpr[:, 8:12])
                nc.sync.dma_start(vp_sb[:, 12:KI], vpr[:, 12:KI])
                for g in range(1, 4):
                    nc.sync.dma_start(kt_sb[:, g], ktr[:, g])
                nc.sync.dma_start(lnp_sb[:], lnp.ap())
                nc.sync.dma_start(b1_sb[:], b1t.ap())
            nc.vector.memset(ones_c[:], 1.0)
            nc.vector.memset(ebias[:], EXP_BIAS)
            nc.vector.memset(epst[:], EPS)

            def w1_og(og):
                # w1 streams in og-column slices: FFN1(og) needs exactly one
                # slice (all kc rows of its 512 hidden columns), so arrival
                # order matches consumption order.
                nc.sync.dma_start(
                    w1_sb[:, :, og * 512:(og + 1) * 512],
                    w1t[:, og * 512:(og + 1) * 512].rearrange(
                        "(kc p) m -> p kc m", p=P))

            def w2_fk(j):
                nc.sync.dma_start(
                    w2_sb[:, 4 * j:4 * j + 4],
                    w2t[4 * j * P:(4 * j + 4) * P, :].rearrange(
                        "(f p) m -> p f m", p=P))

            def stream_weights():
                for og in range(4):
                    w1_og(og)
                w2_fk(0)
                nc.sync.dma_start(
                    wot_sb[:], wot8.ap().rearrange("p (j t m) -> p j t m",
                                                   j=4, t=2))
                for og in range(4, 8):
                    w1_og(og)
                    w2_fk(og - 3)
                for j in range(5, 8):
                    w2_fk(j)

            g1c = lambda kc: lnp_sb[:, kc:kc + 1]
            be1c = lambda kc: lnp_sb[:, 8 + kc:9 + kc]
            g2c = lambda kc: lnp_sb[:, 16 + kc:17 + kc]
            be2c = lambda kc: lnp_sb[:, 24 + kc:25 + kc]
            b2c = lambda kc: lnp_sb[:, 32 + kc:33 + kc]

            state = {}

            def scores_half(c, h, half):
                """scores matmuls + exp for one 8-key-chunk half of head h.
                PSUM zero-region rule: one start=True per 2KB bank region.
                After half 1, the relu-softmax floor clamp runs over the whole
                slab, alternating DVE/Pool per step to balance engine load."""
                qs = QSL[c]
                g, bb = h // 4, h % 4
                pp = slice(32 * bb, 32 * bb + 32)
                if half == 0:
                    state[c]["slab"][h] = slabp.tile([P, 16, qs], F8,
                                                     name="slab")
                slab = state[c]["slab"][h]
                s_ps = spool.tile([P, 8, QS], F32, name="s_ps")
                for kc8 in range(8):
                    kc = half * 8 + kc8
                    nc.tensor.matmul(
                        s_ps[:, kc8, 0:qs],
                        kt_sb[pp, g, :, kc * P:(kc + 1) * P],
                        state[c]["qt"][pp, g, :, :],
                        start=(kc8 % 4 == 0), stop=True,
                        perf_mode=DRM, skip_group_check=True,
                        tile_position=(32 * bb, 0))
                hsl = slice(8 * half, 8 * half + 8)
                nc.scalar.activation(slab[:, hsl, :], s_ps[:, :, 0:qs],
                                     AF.Exp, scale=EXP_SCALE, bias=ebias[:])
                if half == 1:
                    nc.vector.tensor_scalar_max(slab[:], slab[:], PFLOOR)

            def ctx_part(c, h):
                """ctx accumulation over keys + stage raw result to SBUF."""
                qs = QSL[c]
                slab = state[c]["slab"].pop(h)
                c_ps = cpool.tile([65, 512], F32, name="c_ps")
                for pj in range(8):
                    nc.tensor.matmul(
                        c_ps[:, 0:qs],
                        vp_sb[:, 2 * pj:2 * pj + 2, h, :],
                        slab[:, 2 * pj:2 * pj + 2, :],
                        start=(pj == 0), stop=(pj == 7),
                        perf_mode=DRM, skip_group_check=True)
                nc.vector.tensor_copy(state[c]["craw"][:, h, :],
                                      c_ps[:, 0:qs])

            def norm_group(c, grp):
                """Normalize heads 8*grp..8*grp+7 of subchunk c into ctx fp8.
                The ones row of vp is 1/8 on the host, so 1/craw[64] already
                carries the 8x ctx scale; normalization is a plain multiply.
                sig=0 keeps partitions aligned (Pool-safe); sig=1 shifts the
                output partition base, which only DVE handles."""
                qs = QSL[c]
                cr = state[c]["craw"]
                hs = slice(8 * grp, 8 * grp + 8)
                inv = bcb.tile([1, 8, qs], BF16, name="inv")
                with nc.allow_low_precision(reason="fp8 attention path"):
                    nc.vector.reciprocal(inv[:], cr[64:65, hs, :])
                invb = bcb.tile([64, 8, qs], BF16, name="invb")
                nc.gpsimd.partition_broadcast(invb[:], inv[:])
                ctx = state[c]["ctx"]
                for sig, eng in ((0, nc.gpsimd), (1, nc.vector)):
                    eng.tensor_mul(
                        ctx[64 * sig:64 * sig + 64, 4 * grp:4 * grp + 4, :],
                        cr[0:64, 8 * grp + sig:8 * grp + 8:2, :],
                        invb[:, sig::2, :])

            def rstd_via_lnexp(dst_rstd, var_ap):
                # rstd = exp(-0.5 * ln(var + eps)); stays in the exp/ln
                # ACT table set so no table reloads are ever needed.
                nc.scalar.activation(dst_rstd, var_ap, AF.Ln, bias=epst[0:1, :])
                nc.scalar.activation(dst_rstd, dst_rstd, AF.Exp, scale=-0.5)

            def layer_norm(y_sb, out_sb, gc, bec, qs):
                """t-layout LN over the 1024 partition-features.
                y_sb [P, 8, qs] bf16 -> out_sb (bf16 or f32)."""
                st = hpool.tile([P, 4, QS], F32, name="h_ps")
                # two accumulation chains on DIFFERENT partitions (0 and 32):
                # PSUM zero regions are per-partition, so both chains can use
                # clean start=True semantics (no lazy-zero assumptions).
                for kc in range(8):
                    nc.tensor.matmul(st[0:1, 0, 0:qs], ones_c[:],
                                     y_sb[:, kc, :],
                                     start=(kc == 0), stop=(kc == 7),
                                     skip_group_check=True)
                for hf in (0, 1):
                    sq = sqp.tile([P, 4, qs], BF16, name="sq")
                    nc.vector.tensor_mul(sq[:], y_sb[:, 4 * hf:4 * hf + 4, :],
                                         y_sb[:, 4 * hf:4 * hf + 4, :])
                    for k4 in range(4):
                        nc.tensor.matmul(st[32:33, 0, 0:qs], ones_c[:],
                                         sq[:, k4, :], start=(hf + k4 == 0),
                                         stop=(hf == 1 and k4 == 3),
                                         skip_group_check=True)
                stats = smallp.tile([1, 2, qs], F32, name="stats")
                stats_bf = smallp.tile([1, 2, qs], BF16, name="stats_bf")
                nc.vector.tensor_scalar_mul(stats[0:1, 0, :],
                                            st[0:1, 0, 0:qs], 1.0 / D)
                nc.vector.tensor_scalar_mul(stats[0:1, 1, :],
                                            st[32:33, 0, 0:qs], 1.0 / D)
                mean = stats[0:1, 0, :]
                nc.vector.tensor_mul(stats_bf[0:1, 0, :], mean, mean)
                nc.vector.tensor_sub(stats[0:1, 1, :], stats[0:1, 1, :],
                                     stats_bf[0:1, 0, :])       # var
                rstd_via_lnexp(stats[0:1, 1, :], stats[0:1, 1, :])
                nc.vector.tensor_mul(stats_bf[0:1, 1, :], mean,
                                     stats[0:1, 1, :])          # ms (bf16)
                nc.vector.tensor_copy(stats_bf[0:1, 0, :], stats[0:1, 1, :])
                for hf in (0, 1):
                    veng = nc.vector
                    rstd_b = bcb.tile([P, 4, qs], BF16, name="rstd_b")
                    ms_b = bcb.tile([P, 4, qs], BF16, name="ms_b")
                    for k4 in range(4):
                        nc.gpsimd.partition_broadcast(rstd_b[:, k4, :],
                                                      stats_bf[0:1, 0, :])
                        nc.gpsimd.partition_broadcast(ms_b[:, k4, :],
                                                      stats_bf[0:1, 1, :])
                    ks = slice(4 * hf, 4 * hf + 4)
                    tmp = sqp.tile([P, 4, qs], BF16, name="sq")
                    veng.tensor_mul(tmp[:], y_sb[:, ks, :], rstd_b[:])
                    if trivial_affine:
                        veng.tensor_sub(out_sb[:, ks, :], tmp[:], ms_b[:])
                    else:
                        nc.vector.tensor_sub(tmp[:], tmp[:], ms_b[:])
                        for k4 in range(4):
                            kc = 4 * hf + k4
                            nc.vector.tensor_scalar(
                                out=out_sb[:, kc, :], in0=tmp[:, k4, :],
                                scalar1=gc(kc), scalar2=bec(kc),
                                op0=OP.mult, op1=OP.add)

            def att_mm_y1(c):
                """Wo projection (fp8 DR) + residual -> y1[c] (bf16)."""
                qs = QSL[c]
                st = state[c]
                a_ps = spool.tile([P, 8, QS], F32, name="s_ps")
                for mc in range(8):
                    for j in range(4):
                        nc.tensor.matmul(
                            a_ps[:, mc, 0:qs],
                            wot_sb[:, j, :, mc * P:(mc + 1) * P],
                            st["ctx"][:, 2 * j:2 * j + 2, :],
                            start=(j == 0 and mc % 4 == 0), stop=(j == 3),
                            perf_mode=DRM, skip_group_check=True)
                y1 = ypool.tile([P, 8, qs], BF16, name="y1")
                nc.vector.scalar_tensor_tensor(
                    out=y1[:], in0=a_ps[:, :, 0:qs], scalar=ATT_INV,
                    in1=st["xts"][:], op0=OP.mult, op1=OP.add)
                st["y1"] = y1

            def ln1(c):
                """LN1 without rstd: with trivial affine and b1=0, the FFN is
                positively homogeneous per token (relu(a*x)=a*relu(x), a>0),
                so y2 = f + z1 = rstd1*(f' + z1') and LN2 cancels the rstd1
                exactly (eps shift ~1e-6 relative). Only the mean is removed.
                Falls back to the full LN when affine params are nontrivial."""
                qs = QSL[c]
                st = state[c]
                z1 = zpool.tile([P, 8, qs], BF16, name="z1")
                if not trivial_affine:
                    layer_norm(st["y1"], z1, g1c, be1c, qs)
                    st["z1"] = z1
                    return
                y = st["y1"]
                stq = hpool.tile([P, 4, QS], F32, name="h_ps")
                for kc in range(8):
                    nc.tensor.matmul(stq[0:1, 0, 0:qs], ones_c[:], y[:, kc, :],
                                     start=(kc == 0), stop=(kc == 7),
                                     skip_group_check=True)
                mu_bf = smallp.tile([1, 1, qs], BF16, name="mu_bf")
                nc.vector.tensor_scalar_mul(mu_bf[0:1, 0, :],
                                            stq[0:1, 0, 0:qs], 1.0 / D)
                mu_b = bcb.tile([P, 4, qs], BF16, name="mu_b")
                for k4 in range(4):
                    nc.gpsimd.partition_broadcast(mu_b[:, k4, :],
                                                  mu_bf[0:1, 0, :])
                for hf in (0, 1):
                    nc.vector.tensor_sub(z1[:, 4 * hf:4 * hf + 4, :],
                                         y[:, 4 * hf:4 * hf + 4, :], mu_b[:])
                st["z1"] = z1

            def ffn_quarter(c, qi):
                """FFN work in 16-matmul quarters: og = qi//4,
                quarter 0/1 = FFN1 halves (+relu), 2/3 = FFN2 halves.
                The caller orders FFN2(og) one og-slot behind FFN1(og+1) so
                the DVE relu latency hides under FFN1 matmuls; h8 is
                double-buffered per og to allow it."""
                qs = QSL[c]
                st = state[c]
                og, qq = qi // 4, qi % 4
                if qq in (0, 1):
                    if qq == 0:
                        st["h_ps"] = hpool.tile([P, 4, QS], F32, name="h_ps")
                    h_ps = st["h_ps"]
                    for mc4 in (2 * qq, 2 * qq + 1):
                        for kc in range(8):
                            nc.tensor.matmul(
                                h_ps[:, mc4, 0:qs],
                                w1_sb[:, kc, og * 512 + mc4 * P:
                                      og * 512 + (mc4 + 1) * P],
                                st["z1"][:, kc, :],
                                start=(mc4 == 0 and kc == 0),
                                stop=(kc == 7), skip_group_check=True)
                    if qq == 1:
                        h8 = hpool8.tile([P, 4, qs], BF16, name="h8")
                        if trivial_affine:
                            nc.vector.tensor_scalar_max(h8[:],
                                                        h_ps[:, :, 0:qs], 0.0)
                        else:
                            for mc4 in range(4):
                                nc.scalar.activation(
                                    h8[:, mc4, :], h_ps[:, mc4, 0:qs],
                                    AF.Relu,
                                    bias=b1_sb[:, og * 4 + mc4:
                                               og * 4 + mc4 + 1])
                        st["h8"][og] = h8
                else:
                    if og == 0 and qq == 2:
                        st["f_ps"] = fpool.tile([P, 8, QS], F32, name="f_ps",
                                                tag="f_ps")
                    f_ps = st["f_ps"]
                    for i in (2 * (qq - 2), 2 * (qq - 2) + 1):
                        fk = og * 4 + i
                        for mc in range(8):
                            nc.tensor.matmul(
                                f_ps[:, mc, 0:qs],
                                w2_sb[:, fk, mc * P:(mc + 1) * P],
                                st["h8"][og][:, i, :],
                                start=(fk == 0 and mc % 4 == 0),
                                stop=(fk == 31), skip_group_check=True)
                    if qq == 3:
                        del st["h8"][og]

            def finish_a(c):
                """y2 = f + (b2) + z1 and LN2 stats through var - no ACT op.
                The ACT rstd goes in finish_b, enqueued a job later, so it
                never head-of-line-blocks the exp stream while var is still
                in flight."""
                qs = QSL[c]
                st = state[c]
                y2 = ypool.tile([P, 8, qs], BF16, name="y1")
                if trivial_affine:
                    nc.vector.tensor_add(y2[:], st["f_ps"][:, :, 0:qs],
                                         st["z1"][:])
                else:
                    for mc in range(8):
                        nc.vector.scalar_tensor_tensor(
                            out=y2[:, mc, :], in0=st["f_ps"][:, mc, 0:qs],
                            scalar=b2c(mc), in1=st["z1"][:, mc, :],
                            op0=OP.add, op1=OP.add)
                st["y2"] = y2
                stq = hpool.tile([P, 4, QS], F32, name="h_ps")
                for kc in range(8):
                    nc.tensor.matmul(stq[0:1, 0, 0:qs], ones_c[:],
                                     y2[:, kc, :],
                                     start=(kc == 0), stop=(kc == 7),
                                     skip_group_check=True)
                for hf in (0, 1):
                    sq = sqp.tile([P, 4, qs], BF16, name="sq")
                    nc.vector.tensor_mul(sq[:], y2[:, 4 * hf:4 * hf + 4, :],
                                         y2[:, 4 * hf:4 * hf + 4, :])
                    for k4 in range(4):
                        nc.tensor.matmul(stq[32:33, 0, 0:qs], ones_c[:],
                                         sq[:, k4, :], start=(hf + k4 == 0),
                                         stop=(hf == 1 and k4 == 3),
                                         skip_group_check=True)
                stats = smallp.tile([1, 2, qs], F32, name="stats")
                stats_bf = smallp.tile([1, 2, qs], BF16, name="stats_bf")
                nc.vector.tensor_scalar_mul(stats[0:1, 0, :],
                                            stq[0:1, 0, 0:qs], 1.0 / D)
                nc.vector.tensor_scalar_mul(stats[0:1, 1, :],
                                            stq[32:33, 0, 0:qs], 1.0 / D)
                mean = stats[0:1, 0, :]
                nc.vector.tensor_mul(stats_bf[0:1, 0, :], mean, mean)
                nc.vector.tensor_sub(stats[0:1, 1, :], stats[0:1, 1, :],
                                     stats_bf[0:1, 0, :])       # var
                st["ln2"] = (stats, stats_bf)

            def finish_b(c):
                """rstd (ACT), normalize, write out subchunk c."""
                qs = QSL[c]
                st = state[c]
                y2 = st["y2"]
                stats, stats_bf = st["ln2"]
                mean = stats[0:1, 0, :]
                rstd_via_lnexp(stats[0:1, 1, :], stats[0:1, 1, :])
                nc.vector.tensor_mul(stats_bf[0:1, 1, :], mean,
                                     stats[0:1, 1, :])          # ms (bf16)
                nc.vector.tensor_copy(stats_bf[0:1, 0, :], stats[0:1, 1, :])
                for hf in (0, 1):
                    rstd_b = bcb.tile([P, 4, qs], BF16, name="rstd_b")
                    ms_b = bcb.tile([P, 4, qs], BF16, name="ms_b")
                    for k4 in range(4):
                        nc.gpsimd.partition_broadcast(rstd_b[:, k4, :],
                                                      stats_bf[0:1, 0, :])
                        nc.gpsimd.partition_broadcast(ms_b[:, k4, :],
                                                      stats_bf[0:1, 1, :])
                    ks = slice(4 * hf, 4 * hf + 4)
                    z2 = outp.tile([P, 4, qs], F32, name="z2")
                    tmp = sqp.tile([P, 4, qs], BF16, name="sq")
                    nc.vector.tensor_mul(tmp[:], y2[:, ks, :], rstd_b[:])
                    if trivial_affine:
                        nc.vector.tensor_sub(z2[:], tmp[:], ms_b[:])
                    else:
                        nc.vector.tensor_sub(tmp[:], tmp[:], ms_b[:])
                        for k4 in range(4):
                            kc = 4 * hf + k4
                            nc.vector.tensor_scalar(
                                out=z2[:, k4, :], in0=tmp[:, k4, :],
                                scalar1=g2c(kc), scalar2=be2c(kc),
                                op0=OP.mult, op1=OP.add)
                    nc.sync.dma_start(
                        out_t[4 * hf * P:4 * hf * P + 4 * P,
                              OFF[c]:OFF[c] + qs].rearrange(
                            "(kc p) q -> p kc q", p=P), z2[:])
                del state[c]

            # ---- global software-pipelined stream over 64 head-steps ----
            # PE emission per step: ctx(step-3) | scores half 0 | ~1.2us of
            # queued jobs | scores half 1 | rest of the job quantum. The
            # scores halves single-buffer through one 2-bank PSUM slot; the
            # job quantum between them covers the exp WAR latency. Boundary
            # work (norms, Wo+LN1, FFN quarters, LN2+out) drains from the job
            # queue paced by estimated PE-ns so the exp stream never starves.
            jobs = []

            def drain(budget_ns, keep_min=0):
                spent, n = 0, 0
                while len(jobs) > keep_min and spent < budget_ns and n < 8:
                    cost, fn = jobs.pop(0)
                    fn()
                    spent += cost
                    n += 1

            def load_inputs(c, eng=None):
                # later subchunks load via the Pool SWDGE queue so they are
                # not stuck behind the bulk w1/w2 stream on the sync queue
                qs = QSL[c]
                eng = eng or nc.sync
                xts_sb = xpool.tile([P, 8, qs], BF16, name="xts")
                eng.dma_start(
                    xts_sb[:],
                    xts[:, OFF[c]:OFF[c] + qs].rearrange(
                        "(kc p) q -> p kc q", p=P))
                qt_c = xpool.tile([P, 4, 2, qs], F8, name="qt")
                eng.dma_start(
                    qt_c[:],
                    qt8.ap().rearrange("p (g t q) -> p g t q", g=4, t=2)[
                        :, :, :, OFF[c]:OFF[c] + qs])
                state[c] = {
                    "qt": qt_c,
                    "ctx": ctxp.tile([P, 8, qs], F8, name="ctx"),
                    "craw": craw.tile([65, H, qs], BF16, name="craw"),
                    "xts": xts_sb,
                    "slab": {},
                    "h8": {},
                }

            def enqueue_ffn(cp):
                # FFN2(og) trails FFN1(og+1) so relu latency hides
                order = []
                for og in range(8):
                    order += [4 * og, 4 * og + 1]
                    if og >= 1:
                        order += [4 * (og - 1) + 2, 4 * (og - 1) + 3]
                order += [30, 31]
                jc = int(860 * QSL[cp] / 128)
                for n_q, qi in enumerate(order):
                    jobs.append(
                        (jc, lambda cp=cp, qi=qi: ffn_quarter(cp, qi)))
                    if n_q == 1 and cp >= 1:
                        # rstd of LN2(cp-1): two quarters after finish_a so
                        # var is computed before the Ln lands on ACT
                        jobs.append((200, lambda cp=cp: finish_b(cp - 1)))
                jobs.append((jc, lambda cp=cp: finish_a(cp)))

            load_inputs(0)
            load_persist()
            load_inputs(1)
            stream_weights()

            NSTEP = NSUB * H
            for s in range(NSTEP + 3):
                if s >= 3:
                    cp, hp = divmod(s - 3, H)
                    ctx_part(cp, hp)
                    if hp == 7:
                        jobs.insert(0, (0, lambda cp=cp: norm_group(cp, 0)))
                    elif hp == 15:
                        norm_group(cp, 1)
                        # att+ln1 jump the queue so z1(cp) is ready before
                        # ffn(cp) quarters drain; att's a_ps lives in a spool
                        # slot so it does not wait on finish_a(cp-1)'s f_ps
                        jobs.insert(0, (900, lambda cp=cp: att_mm_y1(cp)))
                        jobs.insert(min(3, len(jobs)),
                                    (900, lambda cp=cp: ln1(cp)))
                        enqueue_ffn(cp)
                if s < NSTEP:
                    c, h = divmod(s, H)
                    drain(1600)
                    scores_half(c, h, 0)
                    scores_half(c, h, 1)
                    if h == 15 and c + 2 < NSUB:
                        load_inputs(c + 2, eng=nc.gpsimd)
                else:
                    drain(2400)
            while jobs:
                drain(10**9)
            finish_b(NSUB - 1)
    _pin_act_tables(nc)
    nc.compile()
    return nc


def _pin_act_tables(nc):
    """Restrict the act-table-load pass to the one set that covers every
    ACT function this kernel uses (exp, ln, relu, copy), so exactly one
    table load is emitted instead of per-transition reloads."""
    import bass_rust as _bass_rust
    from concourse.hw_specs import get_activation_tables
    tabs = get_activation_tables(nc.m.arch)
    # keep every entry (act_func_set_id is the index into act_info.json's
    # list) but empty all other sets so the pass can only pick this one.
    only = [(k, (v if k == "natural_log_exp_and_others" else set()))
            for k, v in tabs.items()]
    if any(v for _, v in only):
        nc.insert_act_table_loads = (
            lambda: _bass_rust.insert_act_table_loads(nc, only))


def _get(name, builder, *args):
    if name not in _CACHE:
        _CACHE[name] = builder(*args)
    return _CACHE[name]


def _qcols(c):
    h0, h1 = 2 * c, 2 * c + 1
    r = np.arange(32)
    return np.concatenate([h0 * 64 + r, h1 * 64 + r,
                           h0 * 64 + 32 + r, h1 * 64 + 32 + r])


def _vcols(c):
    h0, h1 = 2 * c, 2 * c + 1
    r = np.arange(64)
    return np.concatenate([h0 * 64 + r, h1 * 64 + r])


def kernel(X, Wq, Wk, Wo, ln1_g, ln1_b, ln2_g, ln2_b, W1, b1, W2, b2):
    f32 = lambda a: np.asarray(a, np.float32)
    X, Wq, Wk, Wo, W1, W2 = map(f32, (X, Wq, Wk, Wo, W1, W2))
    ln1_g, ln1_b, ln2_g, ln2_b, b1, b2 = map(
        f32, (ln1_g, ln1_b, ln2_g, ln2_b, b1, b2))
    Xf = X.reshape(N, D)
    Xt8 = np.ascontiguousarray(Xf.T).astype(E4)          # [D, N]
    WqT, WkT, WoT = Wq.T, Wk.T, Wo.T

    # ---------------- phase A ----------------
    nc_a = _get("a", _build_phase_a)

    def _pretile(arr):   # [D, P] -> [p, (j t m)] with row (j*2+t)*128+p
        return np.ascontiguousarray(
            arr.reshape(4, 2, P, P).transpose(2, 0, 1, 3).reshape(P, 8 * P))

    in_a = []
    for c in range(N_CORES):
        qc, vc = _qcols(c), _vcols(c)
        in_a.append({
            "xt8": Xt8,
            "wq8": _pretile((WqT[:, qc] * WSC).astype(E4)),
            "wk8": _pretile((WkT[:, qc] * WSC).astype(E4)),
            "wv8": _pretile((WoT[:, vc] * WSC).astype(E4)),
        })
    res_a = run_bass_kernel_spmd(nc_a, in_a, core_ids=list(range(N_CORES)))

    # assemble full q/k/v byte arrays [H, DH, N] (uint8 views of e4m3)
    u8 = np.uint8
    qfull = np.empty((H, DH, N), u8)
    kfull = np.empty((H, DH, N), u8)
    vfull = np.empty((H, DH, N), u8)
    for c in range(N_CORES):
        r = res_a.results[c]
        qa = np.asarray(r["q8"]).view(u8).reshape(2, 2, 32, N)  # [i, j, r, :]
        ka = np.asarray(r["k8"]).view(u8).reshape(2, 2, 32, N)
        va = np.asarray(r["v8"]).view(u8).reshape(2, DH, N)     # [j, dh, :]
        for j in range(2):
            qfull[2 * c + j] = qa[:, j].reshape(DH, N)
            kfull[2 * c + j] = ka[:, j].reshape(DH, N)
            vfull[2 * c + j] = va[j]

    # ---------------- phase B host layouts ----------------
    trivial = (not b1.any()) and (not b2.any()) and \
        np.all(ln1_g == 1) and (not ln1_b.any()) and \
        np.all(ln2_g == 1) and (not ln2_b.any())
    nc_b = _get("b", _build_phase_b, trivial)

    w1t = np.ascontiguousarray(W1.astype(BF).T)          # [D, FF] bf16
    w2t = np.ascontiguousarray(W2.astype(BF).T)          # [FF, D] bf16
    one8 = np.float32(1.0 / CTX_SCALE).astype(E4).view(u8)  # ones row = 1/8
    wot_arr = (WoT * WSC).astype(E4).view(u8)            # [D, D]
    # wot8[p, j, t, m] = WoT*16[64*(4j+2t+p//64)+p%64, m]
    wot8 = wot_arr.reshape(4, 2, 2, 64, D).transpose(2, 3, 0, 1, 4) \
        .reshape(P, 4 * 2 * D).copy()
    lnp = np.zeros((P, 40), np.float32)
    for i, vvec in enumerate((ln1_g, ln1_b, ln2_g, ln2_b, b2)):
        lnp[:, 8 * i:8 * i + 8] = vvec.reshape(8, P).T
    b1t = np.ascontiguousarray(b1.reshape(32, P).T)

    in_b = []
    for c in range(N_CORES):
        toks = slice(c * QC, (c + 1) * QC)
        bb = (c * QC) // S
        keys = slice(bb * S, (bb + 1) * S)
        # qt8 [128, 4, 2, QC]: [bb*32+r, g, t, q] = qfull[4g+bb, 32t+r, tok]
        qx = qfull[:, :, toks].reshape(4, 4, 2, 32, QC)   # [g, bb, t, r, q]
        qt8 = qx.transpose(1, 3, 0, 2, 4).reshape(P, 4 * 2 * QC).copy()
        kx = kfull[:, :, keys].reshape(4, 4, 2, 32, S)
        kt8 = kx.transpose(1, 3, 0, 2, 4).reshape(P, 4 * 2 * S).copy()
        # vp8 [128, kc, h, 65]
        vv = vfull[:, :, keys].reshape(H, DH, KI, P).transpose(3, 2, 0, 1)
        vp8 = np.concatenate(
            [vv, np.full((P, KI, H, 1), one8, u8)], axis=3) \
            .reshape(P, KI * H * 65).copy()
        in_b.append({
            "qt8": qt8.view(E4), "kt8": kt8.view(E4), "vp8": vp8.view(E4),
            "wot8": wot8.view(E4),
            "w1t": w1t, "w2t": w2t,
            "xts": np.ascontiguousarray(Xf.T[:, toks]).astype(BF),
            "lnp": lnp, "b1t": b1t,
        })
    res_b = run_bass_kernel_spmd(nc_b, in_b, core_ids=list(range(N_CORES)))
    out_t = np.concatenate(
        [np.asarray(res_b.results[c]["out_t"]) for c in range(N_CORES)],
        axis=1)                                          # [D, N]
    return np.ascontiguousarray(out_t.T).reshape(B, S, D).astype(np.float32)

